# revision 1
# baseline (speedup 1.0000x reference)
"""Trainium2 Bass kernel for sparse-attention (nn_DMA_14903536517676).

Full (unsharded) inputs in, full output out. Internally shards across 8
NeuronCores: data-parallel over batch (B=2) x tensor-parallel over heads
(16 heads -> 4 per core). Per-head dynamic (kth-threshold) mask and the
degenerate-row fixup are folded into the softmax exp bias host-side; the
device runs projections, causal sparse attention, and the per-core o_proj
partial, which the host reduces.

Matmuls run as float32r (full-rate fp32-replicated mode, ~1e-4 worst-case
rounding on engine-produced operands, ~1e-6 on DMA-fed ones).
"""

import math

import numpy as np

import concourse.bass as bass
import concourse.mybir as mybir
import concourse.tile as tile
from concourse.bass_utils import run_bass_kernel_spmd

F32 = mybir.dt.float32
F32R = mybir.dt.float32r
AF = mybir.ActivationFunctionType
OP = mybir.AluOpType

B, S, D, H = 2, 2048, 2048, 16
P = 128
DH = D // H            # 128, == one partition tile per head
NCORE = 8
HGRP = NCORE // B      # 4 head-groups (cores) per batch element
HPC = H // HGRP        # 4 heads per core
HD = HPC * DH          # 512 head dims per core
NKT = D // P           # 16 contraction tiles over D
SCH = 256              # stage-A s-chunk width
NSCH = S // SCH        # 8
QCH = 512              # stage-B q-chunk width
NQCH = S // QCH        # 4
NST = S // P           # 16 s-tiles
MIN32 = float(np.finfo(np.float32).min)
NEG = -1.0e37          # clamped mask sentinel (exp underflows to exact 0)
INV_SQRT_DH = 1.0 / math.sqrt(DH)

_prog_cache = {}


# ---------------------------------------------------------------------------
# Walrus in this toolchain accepts at most ONE embedded sync-wait command per
# instruction. Tile emits more. Move overflow waits onto InstNoOp
# instructions inserted immediately before, on the same engine (semaphores
# are monotonic in this loop-free program, so waiting earlier is safe).
# ---------------------------------------------------------------------------
def _fix_waits(nc, maxw=1):
    uid = 0
    for f in nc.m.functions:
        for b in f.blocks:
            il = b.instructions
            idx = 0
            while idx < len(il):
                inst = il[idx]
                si = getattr(inst, "sync_info", None)
                if si is None:
                    idx += 1
                    continue
                waits = si.on_wait or []
                if len(waits) <= maxw:
                    idx += 1
                    continue
                si.on_wait = waits[-maxw:]
                overflow = waits[:-maxw]
                nops = []
                for j in range(0, len(overflow), maxw):
                    uid += 1
                    nop = mybir.InstNoOp(name=f"I-waitnop-{uid}")
                    nop.engine = inst.engine
                    nop.sync_info = mybir.SyncInfo(
                        on_wait=overflow[j:j + maxw], on_update=[])
                    nops.append(nop)
                for k, nop in enumerate(nops):
                    il.insert(idx + k, nop)
                idx += len(nops) + 1


def _build_program():
    nc = bass.Bass()

    xt_d = nc.declare_dram_parameter("xt", [NSCH, P, NKT, SCH], F32R, isOutput=False)
    wq_d = nc.declare_dram_parameter("wq", [HPC, NKT, P, P], F32R, isOutput=False)
    wk_d = nc.declare_dram_parameter("wk", [HPC, NKT, P, P], F32R, isOutput=False)
    wv_d = nc.declare_dram_parameter("wv", [NKT, P, HD], F32R, isOutput=False)
    wo_d = nc.declare_dram_parameter("wo", [HPC, P, D], F32R, isOutput=False)
    am_d = nc.declare_dram_parameter("am", [NQCH, 4, P, QCH], F32, isOutput=False)
    tp_d = nc.declare_dram_parameter("tp", [HPC, P, NKT], F32, isOutput=False)
    vm_d = nc.declare_dram_parameter("vm", [P, HPC], F32, isOutput=False)
    ones_d = nc.declare_dram_parameter("onesw", [P, P], F32R, isOutput=False)
    bq_d = nc.declare_dram_parameter("bqv", [P, HPC], F32, isOutput=False)
    bk_d = nc.declare_dram_parameter("bkv", [P, HPC], F32, isOutput=False)
    bvb_d = nc.declare_dram_parameter("bvb", [P, HD], F32, isOutput=False)
    out_d = nc.declare_dram_parameter("opart", [S, D], F32, isOutput=True)

    # internal DRAM scratch between projection and attention stages
    sq_d = nc.dram_tensor("scratch_q", [HPC, P, S], F32R)
    sk_d = nc.dram_tensor("scratch_k", [HPC, P, S], F32R)
    sv_d = nc.dram_tensor("scratch_v", [NST, P, HD], F32R)

    with tile.TileContext(nc) as tc:
        # Small stage-B constants live in the outermost pool (lowest SBUF
        # addresses) so their loads can run during stage A instead of
        # waiting for the stage-A pools to release their address range.
        pre_cm = tc.tile_pool(name="pre", bufs=1)
        pre = pre_cm.__enter__()
        am_sb = pre.tile([P, NQCH * 4, QCH], F32)
        ones_sb = pre.tile([P, P], F32R)
        vm_sb = pre.tile([P, HPC], F32)
        tpa_sb = pre.tile([P, HPC, NKT], F32)
        # head 0's attention inputs: preallocated outside the stage-A pools
        # so their loads start as soon as the scratch data is written, not
        # when stage A's SBUF range drains
        qh0_sb = pre.tile([P, S], F32R)
        kh0_sb = pre.tile([P, S], F32R)
        vh0_sb = pre.tile([P, NKT, P], F32R)

        # ---------------- Stage A: Q/K/V projections -----------------
        with tc.tile_pool(name="aw", bufs=1) as aw, \
             tc.tile_pool(name="ax", bufs=2) as ax, \
             tc.tile_pool(name="acp", bufs=3) as acp, \
             tc.tile_pool(name="apq", bufs=3, space="PSUM") as apq, \
             tc.tile_pool(name="apv", bufs=2, space="PSUM") as apv:
            wq_sb = aw.tile([P, HPC, NKT, P], F32R)
            wk_sb = aw.tile([P, HPC, NKT, P], F32R)
            wv_sb = aw.tile([P, NKT, HD], F32R)
            bq_sb = aw.tile([P, HPC], F32)
            bk_sb = aw.tile([P, HPC], F32)
            bvb_sb = aw.tile([P, HD], F32)
            # dep-free PE warmup: reads an uninitialized tile, output never
            # consumed — only the PE activity matters
            dmy = aw.tile([P, 5 * P], F32R)
            nc.vector.memset(dmy.bitcast(F32)[:], 1.0)
            for _ in range(8):
                pdmy = apq.tile([P, QCH], F32, tag="pq", name="pdmy")
                nc.tensor.matmul(pdmy[:], dmy[:, :P], dmy[:, P:],
                                 start=True, stop=True)
            # Queue plan: sync = wq[0] then the xt strips (the critical path
            # for the first matmuls); gpsimd = remaining weights in the order
            # compute consumes them (q tiles, then k, then v); scalar stays
            # free for the PSUM->SBUF copies + scratch writes.
            nc.scalar.dma_start(bq_sb[:], bq_d[:])
            nc.sync.dma_start(wq_sb[:, 0, :NKT // 2],
                              wq_d[0][:NKT // 2].rearrange("k p m -> p k m"))
            nc.scalar.dma_start(wq_sb[:, 0, NKT // 2:],
                                wq_d[0][NKT // 2:].rearrange("k p m -> p k m"))
            # gpsimd: wq1-3 + wk0 in consumption order, then wv in fine
            # slices (chunk-0's v matmuls consume kt ascending); wk1-3 ride
            # the sync queue between early xt strips (sync has slack there)
            for hd in range(1, HPC):
                nc.gpsimd.dma_start(
                    wq_sb[:, hd], wq_d[hd].rearrange("k p m -> p k m"))
            nc.gpsimd.dma_start(
                wk_sb[:, 0], wk_d[0].rearrange("k p m -> p k m"))
            for sl in range(8):
                nc.gpsimd.dma_start(
                    wv_sb[:, sl * 2:(sl + 1) * 2],
                    wv_d[sl * 2:(sl + 1) * 2].rearrange("k p n -> p k n"))
            nc.scalar.dma_start(bk_sb[:], bk_d[:])
            nc.scalar.dma_start(bvb_sb[:], bvb_d[:])

            def emit_q(c, hd, xt):
                pq = apq.tile([P, SCH], F32, tag="pq", name="pq")
                for kt in range(NKT):
                    nc.tensor.matmul(pq[:], wq_sb[:, hd, kt], xt[:, kt],
                                     start=(kt == 0), stop=(kt == NKT - 1))
                qsb = acp.tile([P, SCH], F32R, tag="qcp", name="qsb")
                nc.scalar.activation(qsb[:], pq[:], AF.Identity,
                                     bias=bq_sb[:, hd:hd + 1])
                nc.scalar.dma_start(sq_d[hd][:, c * SCH:(c + 1) * SCH],
                                    qsb[:])

            def emit_k(c, hd, xt):
                pk = apq.tile([P, SCH], F32, tag="pk", name="pk")
                for kt in range(NKT):
                    nc.tensor.matmul(pk[:], wk_sb[:, hd, kt], xt[:, kt],
                                     start=(kt == 0), stop=(kt == NKT - 1))
                ksb = acp.tile([P, SCH], F32R, tag="kcp", name="ksb")
                nc.scalar.activation(ksb[:], pk[:], AF.Identity,
                                     bias=bk_sb[:, hd:hd + 1])
                nc.scalar.dma_start(sk_d[hd][:, c * SCH:(c + 1) * SCH],
                                    ksb[:])

            def emit_v(c, st, xt):
                pv = apv.tile([P, HD], F32, tag="pv", name="pv")
                for kt in range(NKT):
                    nc.tensor.matmul(pv[:], xt[:, kt, st * P:(st + 1) * P],
                                     wv_sb[:, kt],
                                     start=(kt == 0), stop=(kt == NKT - 1))
                vsb = acp.tile([P, HD], F32R, tag="vcp", name="vsb")
                nc.vector.scalar_tensor_tensor(vsb[:], pv[:], 1.0, bvb_sb[:],
                                               op0=OP.mult, op1=OP.add)
                nc.gpsimd.dma_start(sv_d[c * (SCH // P) + st], vsb[:])

            for c in range(NSCH):
                xt = ax.tile([P, NKT, SCH], F32R, tag="xt")
                if c == 0:
                    nc.sync.dma_start(xt[:, :NKT // 2], xt_d[c][:, :NKT // 2])
                    nc.scalar.dma_start(xt[:, NKT // 2:], xt_d[c][:, NKT // 2:])
                else:
                    nc.sync.dma_start(xt[:], xt_d[c])
                if c == 0:
                    for hd in range(1, HPC):
                        nc.sync.dma_start(
                            wk_sb[:, hd], wk_d[hd].rearrange("k p m -> p k m"))
                if c < NSCH - 1:
                    for hd in range(HPC):
                        emit_q(c, hd, xt)
                    for hd in range(HPC):
                        emit_k(c, hd, xt)
                    for st in range(SCH // P):
                        emit_v(c, st, xt)
                else:
                    # last chunk: v first, q/k per-head interleaved, so the
                    # scratch data head 0 needs lands as early as possible
                    for st in range(SCH // P):
                        emit_v(c, st, xt)
                    for hd in range(HPC):
                        emit_q(c, hd, xt)
                        emit_k(c, hd, xt)
                if c == 2:
                    for j in range(NQCH):
                        nc.sync.dma_start(am_sb[:, j * 4:(j + 1) * 4],
                                          am_d[j].rearrange("d p q -> p d q"))
                    nc.sync.dma_start(ones_sb[:], ones_d[:])
                    nc.sync.dma_start(vm_sb[:], vm_d[:])
                    nc.sync.dma_start(tpa_sb[:], tp_d.rearrange("h p k -> p h k"))

        # ---------------- Stage B: attention ------------------------
        with tc.tile_pool(name="bfix", bufs=1) as bfix, \
             tc.tile_pool(name="bot", bufs=1) as bot:
            ot_sb = bot.tile([P, HPC, S], F32R)
            wo_sb = bfix.tile([P, HPC, D], F32R)

            with tc.tile_pool(name="bh", bufs=2) as bh, \
                 tc.tile_pool(name="be", bufs=3) as be, \
                 tc.tile_pool(name="bt", bufs=2) as bt, \
                 tc.tile_pool(name="bps", bufs=5, space="PSUM") as bps, \
                 tc.tile_pool(name="bpo", bufs=2, space="PSUM") as bpo, \
                 tc.tile_pool(name="bpr", bufs=1, space="PSUM") as bpr:
                # Software-pipelined with a 1-iteration skew: the scores
                # matmul of iteration i+1 is emitted BEFORE the PV/rowsum
                # matmuls of iteration i, so the PE (in-order stream) never
                # stalls on the exp between scores and PV.
                def emit_pv(st_):
                    po_, pr_, vh_, ev_, kt_, first_, last_, off_ = st_
                    # diag tiles contribute exact zeros in E[:, :off]; the
                    # group's first (full-width) matmul has already cleared
                    # the psum there, so the accumulation can skip it
                    nc.tensor.matmul(po_[:, off_:], vh_[:, kt_], ev_[:, off_:],
                                     start=first_, stop=last_)
                    nc.tensor.matmul(pr_[:, off_:], ones_sb[:], ev_[:, off_:],
                                     start=first_, stop=last_)

                def emit_epilogue(ep_):
                    h_, j_, po_, pr_ = ep_
                    flag = bt.tile([P, QCH], F32, tag="flag")
                    nc.vector.tensor_scalar(flag[:], pr_[:], 0.0, None,
                                            op0=OP.is_equal)
                    rs2 = bt.tile([P, QCH], F32, tag="rs2")
                    nc.vector.tensor_tensor(rs2[:], pr_[:], flag[:],
                                            op=OP.add)
                    recip = bt.tile([P, QCH], F32, tag="recip")
                    nc.vector.reciprocal(recip[:], rs2[:])
                    o1 = bt.tile([P, QCH], F32, tag="o1")
                    nc.vector.tensor_tensor(o1[:], po_[:], recip[:],
                                            op=OP.mult)
                    nc.vector.scalar_tensor_tensor(
                        ot_sb[:, h_, j_ * QCH:(j_ + 1) * QCH], flag[:],
                        vm_sb[:, h_:h_ + 1], o1[:], op0=OP.mult, op1=OP.add)

                pending = None        # skewed (po, pr, vh, ev, kt, ...)
                pending_ep = None     # epilogue of the finished (h, j)
                for h in range(HPC):
                    if h == 0:
                        qh, kh, vh = qh0_sb, kh0_sb, vh0_sb
                        nc.sync.dma_start(qh[:], sq_d[h])
                        nc.sync.dma_start(kh[:], sk_d[h])
                        nc.gpsimd.dma_start(
                            vh[:], sv_d[:, :, h * DH:(h + 1) * DH]
                            .rearrange("k p m -> p k m"))
                    else:
                        qh = bh.tile([P, S], F32R, tag="qh")
                        nc.sync.dma_start(qh[:], sq_d[h])
                        kh = bh.tile([P, S], F32R, tag="kh")
                        nc.gpsimd.dma_start(kh[:], sk_d[h])
                        vh = bh.tile([P, NKT, P], F32R, tag="vh")
                        nc.gpsimd.dma_start(
                            vh[:], sv_d[:, :, h * DH:(h + 1) * DH]
                            .rearrange("k p m -> p k m"))
                    if h == 2:
                        # o_proj weights: late enough not to starve the
                        # qh/kh loads, early enough to cover stage C
                        for hh in range(HPC):
                            [nc.sync, nc.gpsimd, nc.sync, nc.gpsimd][hh] \
                                .dma_start(wo_sb[:, hh], wo_d[hh])
                    for j in range(NQCH):
                        nkt = 4 * j + 4
                        po = bpo.tile([P, QCH], F32, tag="po")
                        pr = bpr.tile([P, QCH], F32, tag="pr")
                        for kt in range(nkt):
                            ps = bps.tile([P, QCH], F32, tag="ps")
                            # diag tiles: columns q < dkt*128 are fully
                            # causal-masked (the am add + exp zero them), so
                            # the scores matmul skips them. Keep width >=256
                            # (fp32r fast regime) and write full tiles while
                            # the PSUM slots are virgin (h0/j0).
                            off = 0
                            if kt >= 4 * j and not (h == 0 and j == 0):
                                off = min((kt - 4 * j) * P, QCH - 2 * P)
                            nc.tensor.matmul(
                                ps[:, off:],
                                kh[:, kt * P:(kt + 1) * P],
                                qh[:, j * QCH + off:(j + 1) * QCH],
                                start=True, stop=True)
                            if pending is not None:
                                emit_pv(pending)
                                pending = None
                            if pending_ep is not None:
                                emit_epilogue(pending_ep)
                                pending_ep = None
                            if kt >= 4 * j:
                                # masked (k > q) entries only occupy the
                                # first (dkt+1)*128 columns of a diag tile
                                w = (kt - 4 * j + 1) * P
                                nc.vector.tensor_tensor(
                                    ps[:, :w], ps[:, :w],
                                    am_sb[:, j * 4 + (kt - 4 * j), :w],
                                    op=OP.add)
                            ev = be.tile([P, QCH], F32R, tag="ev")
                            nc.scalar.activation(ev[:], ps[:], AF.Exp,
                                                 bias=tpa_sb[:, h, kt:kt + 1],
                                                 scale=INV_SQRT_DH)
                            pv_off = off if kt > 0 else 0
                            pending = (po, pr, vh, ev, kt,
                                       kt == 0, kt == nkt - 1, pv_off)
                        pending_ep = (h, j, po, pr)
                if pending is not None:
                    emit_pv(pending)
                if pending_ep is not None:
                    emit_epilogue(pending_ep)

            # ---------------- Stage C: o_proj partial ----------------
            with tc.tile_pool(name="ccp", bufs=3) as ccp, \
                 tc.tile_pool(name="cps", bufs=4, space="PSUM") as cps:
                for st in range(NST):
                    for ec in range(D // QCH):
                        pc = cps.tile([P, QCH], F32, tag="pc")
                        for h in range(HPC):
                            nc.tensor.matmul(
                                pc[:], ot_sb[:, h, st * P:(st + 1) * P],
                                wo_sb[:, h, ec * QCH:(ec + 1) * QCH],
                                start=(h == 0), stop=(h == HPC - 1))
                        osb = ccp.tile([P, QCH], F32, tag="osb")
                        nc.scalar.copy(osb[:], pc[:])
                        (nc.sync if ec % 2 == 0 else nc.gpsimd).dma_start(
                            out_d[st * P:(st + 1) * P,
                                  ec * QCH:(ec + 1) * QCH], osb[:])
        pre_cm.__exit__(None, None, None)

    _fix_waits(nc, 1)
    return nc


def _host_mask_and_vmean(hidden_states, Wv, bv, Wdt, bdt, A, ratio_permille):
    """Dynamic-mask pipeline on host, bit-matched to the jax reference."""
    import jax
    import jax.numpy as jnp

    cpu = jax.devices("cpu")[0]
    with jax.default_device(cpu):
        hs = jnp.asarray(hidden_states, dtype=jnp.float32)
        v_lin = jnp.einsum('bsd,ed->bse', hs, jnp.asarray(Wv, jnp.float32)) \
            + jnp.asarray(bv, jnp.float32)
        dt = jnp.einsum('bsd,hd->bsh', v_lin, jnp.asarray(Wdt, jnp.float32)) \
            + jnp.asarray(bdt, jnp.float32)
        dyn = jnp.exp(jnp.asarray(A, jnp.float32) * jax.nn.softplus(dt))
        dynT = dyn.transpose(0, 2, 1)                       # [B, H, S]
        ratio = float(ratio_permille) / 1000.0
        num = int(S * ratio)
        if 0.0 < ratio < 1.0 and num > 0:
            kth = jnp.sort(dynT, axis=-1)[..., num - 1:num]
            tmask = jnp.where(dynT < kth, NEG, dynT)
        else:
            tmask = dynT
        vmean = v_lin.mean(axis=1)                          # [B, D]
        tmask = np.asarray(tmask, dtype=np.float32)
        vmean = np.asarray(vmean, dtype=np.float32)
    return np.maximum(tmask, np.float32(NEG)), vmean


def kernel(hidden_states, attention_mask, Wq, bq, Wk, bk, Wv, bv,
           Wdt, bdt, A, Wo, bo, ratio_permille):
    f32 = np.float32
    hidden_states = np.asarray(hidden_states, f32)
    attention_mask = np.asarray(attention_mask, f32)
    Wq, bq = np.asarray(Wq, f32), np.asarray(bq, f32)
    Wk, bk = np.asarray(Wk, f32), np.asarray(bk, f32)
    Wv, bv = np.asarray(Wv, f32), np.asarray(bv, f32)
    Wdt, bdt = np.asarray(Wdt, f32), np.asarray(bdt, f32)
    A_, Wo, bo = np.asarray(A, f32), np.asarray(Wo, f32), np.asarray(bo, f32)

    tmask, vmean = _host_mask_and_vmean(hidden_states, Wv, bv, Wdt, bdt, A_,
                                        ratio_permille)
    amc = np.where(attention_mask[:, 0] == np.float32(MIN32),
                   f32(NEG), f32(0.0)).astype(f32)          # [B, S, S] (q, k)

    ones_blk = np.ones((P, P), f32)
    in_maps = []
    for c in range(NCORE):
        b, hg = divmod(c, HGRP)
        h0 = hg * HPC                                        # first global head
        e0 = hg * HD                                         # first head dim

        x = hidden_states[b]                                 # [S, D]
        xt = np.ascontiguousarray(
            x.reshape(NSCH, SCH, NKT, P).transpose(0, 3, 2, 1))
        wq_c = np.ascontiguousarray(
            Wq[e0:e0 + HD].reshape(HPC, P, NKT, P).transpose(0, 2, 3, 1))
        wk_c = np.ascontiguousarray(
            Wk[e0:e0 + HD].reshape(HPC, P, NKT, P).transpose(0, 2, 3, 1))
        wv_c = np.ascontiguousarray(
            Wv[e0:e0 + HD].reshape(HD, NKT, P).transpose(1, 2, 0))
        wo_c = np.ascontiguousarray(
            Wo[:, e0:e0 + HD].T.reshape(HPC, P, D))
        am_c = np.empty((NQCH, 4, P, QCH), f32)
        for j in range(NQCH):
            blk = amc[b, j * QCH:(j + 1) * QCH, j * QCH:(j + 1) * QCH]
            am_c[j] = blk.T.reshape(4, P, QCH)
        tp_c = np.ascontiguousarray(
            tmask[b, h0:h0 + HPC].reshape(HPC, NKT, P).transpose(0, 2, 1))
        vm_c = np.ascontiguousarray(vmean[b, e0:e0 + HD].reshape(HPC, P).T)
        bq_c = np.ascontiguousarray(bq[e0:e0 + HD].reshape(HPC, P).T)
        bk_c = np.ascontiguousarray(bk[e0:e0 + HD].reshape(HPC, P).T)
        bvb_c = np.ascontiguousarray(
            np.broadcast_to(bv[e0:e0 + HD], (P, HD))).astype(f32)

        in_maps.append({
            "xt": xt, "wq": wq_c, "wk": wk_c, "wv": wv_c, "wo": wo_c,
            "am": am_c, "tp": tp_c, "vm": vm_c, "onesw": ones_blk,
            "bqv": bq_c, "bkv": bk_c, "bvb": bvb_c,
        })

    if "nc" not in _prog_cache:
        _prog_cache["nc"] = _build_program()
    nc = _prog_cache["nc"]

    res = run_bass_kernel_spmd(nc, in_maps, list(range(NCORE)))

    out = np.zeros((B, S, D), np.float64)
    for c in range(NCORE):
        b = c // HGRP
        out[b] += res.results[c]["opart"].astype(np.float64)
    out += bo.astype(np.float64)
    return out.astype(f32)



# revision 3
# speedup vs baseline: 1.2787x; 1.2787x over previous
"""Trainium2 Bass kernel for sparse-attention (nn_DMA_14903536517676).

Full (unsharded) inputs in, full output out. Shards across 8 NeuronCores:
data-parallel over batch (B=2) x tensor-parallel over heads (4 per core).

Key idea vs the dense baseline: the dynamic (kth-threshold) mask depends only
on the KEY position, killing ~half of all keys per (batch, head). The host
computes the surviving-key set per head and gathers the corresponding x
columns; the device then computes K/V projections and the whole attention
block only over compacted survivor slots (causal q-chunks touch only a
prefix of the survivor list). All matmul operands are bf16 (full PE rate at
any tile width in fp32-accumulate), Q/K/V stay SBUF-resident (no DRAM
scratch roundtrip), and the o_proj partial is written bf16 and reduced on
host. Dynamic-mask values ride the exp bias per survivor slot; the ragged
causal edge is a small set of host-built 0/1 tiles multiplied into ev.
"""

import math

import numpy as np
import ml_dtypes

import concourse.bass as bass
import concourse.mybir as mybir
import concourse.tile as tile
from concourse.bass_utils import run_bass_kernel_spmd

F32 = mybir.dt.float32
F32R = mybir.dt.float32r
BF16 = mybir.dt.bfloat16
NPBF16 = ml_dtypes.bfloat16
AF = mybir.ActivationFunctionType
OP = mybir.AluOpType

B, S, D, H = 2, 2048, 2048, 16
P = 128
DH = D // H            # 128
NCORE = 8
HGRP = NCORE // B      # 4 head-groups (cores) per batch element
HPC = H // HGRP        # 4 heads per core
HD = HPC * DH          # 512 head dims per core
NKT = D // P           # 16 contraction tiles over D
XCH = 256              # Q-projection s-chunk width
NXC = S // XCH         # 8
QCH = 512              # attention q-chunk width
NQCH = S // QCH        # 4
MIN32 = float(np.finfo(np.float32).min)
NEG = -1.0e37          # clamped mask sentinel (exp underflows to exact 0)
NEGT = -1.0e36         # threshold for "is masked" tests on host
INV_SQRT_DH = 1.0 / math.sqrt(DH)

_prog_cache = {}


# ---------------------------------------------------------------------------
# Walrus in this toolchain accepts at most ONE embedded sync-wait command per
# instruction. Tile emits more. Move overflow waits onto InstNoOp
# instructions inserted immediately before, on the same engine (semaphores
# are monotonic in this loop-free program, so waiting earlier is safe).
# ---------------------------------------------------------------------------
def _fix_waits(nc, maxw=1):
    uid = 0
    for f in nc.m.functions:
        for b in f.blocks:
            il = b.instructions
            idx = 0
            while idx < len(il):
                inst = il[idx]
                si = getattr(inst, "sync_info", None)
                if si is None:
                    idx += 1
                    continue
                waits = si.on_wait or []
                if len(waits) <= maxw:
                    idx += 1
                    continue
                si.on_wait = waits[-maxw:]
                overflow = waits[:-maxw]
                nops = []
                for j in range(0, len(overflow), maxw):
                    uid += 1
                    nop = mybir.InstNoOp(name=f"I-waitnop-{uid}")
                    nop.engine = inst.engine
                    nop.sync_info = mybir.SyncInfo(
                        on_wait=overflow[j:j + maxw], on_update=[])
                    nops.append(nop)
                for k, nop in enumerate(nops):
                    il.insert(idx + k, nop)
                idx += len(nops) + 1


def _chunk_plan(nt):
    """Split nt 128-wide tiles into DMA/compute chunks of 2 (last 3 if odd)."""
    if nt <= 3:
        return [(0, nt)]
    cuts = list(range(0, nt - 3, 2)) + [nt - 3 if nt % 2 else nt - 2]
    cuts = sorted(set(c for c in cuts if c >= 0))
    plan = []
    prev = 0
    for c in cuts[1:] + [nt]:
        plan.append((prev, c))
        prev = c
    return plan


def _build_program(cfg):
    T = cfg["T"]
    NT = cfg["NT"]
    KP = NT * P
    EDGE = cfg["EDGE"]
    JD = set(cfg["JD"])
    edge_list = [(j, kt) for j in range(NQCH) for kt in EDGE[j]]
    NE = max(len(edge_list), 1)
    eidx = {jk: i for i, jk in enumerate(edge_list)}

    nc = bass.Bass()

    xt_d = nc.declare_dram_parameter("xt", [NXC, P, NKT, XCH], BF16, isOutput=False)
    xkg_d = nc.declare_dram_parameter("xkg", [HPC, P, NKT, KP], BF16, isOutput=False)
    wq_d = nc.declare_dram_parameter("wq", [P, HPC, NKT, P], BF16, isOutput=False)
    wk_d = nc.declare_dram_parameter("wk", [P, HPC, NKT, P], BF16, isOutput=False)
    wv_d = nc.declare_dram_parameter("wv", [P, HPC, NKT, P], BF16, isOutput=False)
    wo_d = nc.declare_dram_parameter("wo", [P, HPC, D], BF16, isOutput=False)
    am_d = nc.declare_dram_parameter("am", [HPC, P, NE, QCH], BF16, isOutput=False)
    tpa_d = nc.declare_dram_parameter("tp", [P, HPC, NT], F32, isOutput=False)
    ones_d = nc.declare_dram_parameter("onesw", [P, P], BF16, isOutput=False)
    bq_d = nc.declare_dram_parameter("bqv", [P, HPC], F32, isOutput=False)
    bk_d = nc.declare_dram_parameter("bkv", [P, HPC], F32, isOutput=False)
    bvb_d = nc.declare_dram_parameter("bvb", [P, HD], F32, isOutput=False)
    vm_d = nc.declare_dram_parameter("vm", [P, HPC], F32, isOutput=False)
    out_d = nc.declare_dram_parameter("opart", [S, D], BF16, isOutput=True)

    plan = _chunk_plan(NT)
    WMAX = max(t1 - t0 for t0, t1 in plan) * P

    with tile.TileContext(nc) as tc:
        pre_cm = tc.tile_pool(name="pre", bufs=1)
        pre = pre_cm.__enter__()
        q_sb = pre.tile([P, HPC, S], BF16)
        k_sb = pre.tile([P, HPC, KP], BF16)
        v_sb = pre.tile([P, HPC, NT, P], BF16)
        ot_sb = pre.tile([P, HPC, S], BF16)
        wo_sb = pre.tile([P, HPC, D], BF16)
        ones_sb = pre.tile([P, P], BF16)
        tpa_sb = pre.tile([P, HPC, NT], F32)
        vm_sb = pre.tile([P, HPC], F32)
        bq_sb = pre.tile([P, HPC], F32)
        bk_sb = pre.tile([P, HPC], F32)
        bvb_sb = pre.tile([P, HD], F32)
        pxg_cm = tc.tile_pool(name="pxg", bufs=2)
        pxg = pxg_cm.__enter__()
        amp_cm = tc.tile_pool(name="amp", bufs=2)
        amp = amp_cm.__enter__()

        # small constants early on the scalar queue
        nc.scalar.dma_start(tpa_sb[:], tpa_d[:])
        nc.scalar.dma_start(vm_sb[:], vm_d[:])
        nc.scalar.dma_start(bq_sb[:], bq_d[:])
        nc.scalar.dma_start(bk_sb[:], bk_d[:])
        nc.scalar.dma_start(bvb_sb[:], bvb_d[:])
        nc.scalar.dma_start(ones_sb[:], ones_d[:])

        with tc.tile_pool(name="aw", bufs=1) as aw:
            wk_sb = aw.tile([P, HPC, NKT, P], BF16)
            wv_sb = aw.tile([P, HPC, NKT, P], BF16)

            # ------------- Stage A: Q projection (dense) -------------
            with tc.tile_pool(name="awq", bufs=1) as awq, \
                 tc.tile_pool(name="ax", bufs=2) as ax, \
                 tc.tile_pool(name="apq", bufs=3, space="PSUM") as apq:
                wq_sb = awq.tile([P, HPC, NKT, P], BF16)
                # dep-free PE warmup (ramps the p-state before real work)
                dmy = awq.tile([P, 5 * P], F32R)
                nc.vector.memset(dmy.bitcast(F32)[:], 1.0)
                for _ in range(8):
                    pdmy = apq.tile([P, 4 * P], F32, tag="pq", name="pdmy")
                    nc.tensor.matmul(pdmy[:], dmy[:, :P], dmy[:, P:],
                                     start=True, stop=True)
                nc.scalar.dma_start(wq_sb[:], wq_d[:])
                nc.scalar.dma_start(wk_sb[:], wk_d[:])
                nc.scalar.dma_start(wv_sb[:], wv_d[:])
                # xkg streamed on gpsimd for all heads (chunked); am per head
                xg_tiles = {}
                for hh in range(HPC):
                    for ci, (t0, t1) in enumerate(plan):
                        w = (t1 - t0) * P
                        xg = pxg.tile([P, NKT, WMAX], BF16, tag="xkg")
                        nc.gpsimd.dma_start(
                            xg[:, :, :w], xkg_d[hh][:, :, t0 * P:t1 * P])
                        xg_tiles[(hh, ci)] = xg
                    am_t = amp.tile([P, NE, QCH], BF16, tag="am")
                    nc.gpsimd.dma_start(am_t[:], am_d[hh])
                    xg_tiles[(hh, "am")] = am_t

                for c in range(NXC):
                    xt = ax.tile([P, NKT, XCH], BF16, tag="xt")
                    nc.sync.dma_start(xt[:], xt_d[c])
                    for hh in range(HPC):
                        pq = apq.tile([P, XCH], F32, tag="pq", name="pq")
                        for kt in range(NKT):
                            nc.tensor.matmul(pq[:], wq_sb[:, hh, kt],
                                             xt[:, kt],
                                             start=(kt == 0),
                                             stop=(kt == NKT - 1))
                        nc.scalar.activation(
                            q_sb[:, hh, c * XCH:(c + 1) * XCH], pq[:],
                            AF.Identity, bias=bq_sb[:, hh:hh + 1])
            nc.scalar.dma_start(wo_sb[:], wo_d[:])

            # -------- Stage A2 (K/V compact) + Stage B (attention) ---
            with tc.tile_pool(name="bev", bufs=3) as bev, \
                 tc.tile_pool(name="bt", bufs=2) as bt, \
                 tc.tile_pool(name="apk", bufs=1, space="PSUM") as apk, \
                 tc.tile_pool(name="apv", bufs=1, space="PSUM") as apv, \
                 tc.tile_pool(name="bps", bufs=3, space="PSUM") as bps, \
                 tc.tile_pool(name="bpo", bufs=2, space="PSUM") as bpo, \
                 tc.tile_pool(name="bpr", bufs=1, space="PSUM") as bpr:

                def emit_a2(hh):
                    for ci, (t0, t1) in enumerate(plan):
                        w = (t1 - t0) * P
                        xg = xg_tiles[(hh, ci)]
                        pk = apk.tile([P, WMAX], F32, tag="pk")
                        for kt in range(NKT):
                            nc.tensor.matmul(pk[:, :w], wk_sb[:, hh, kt],
                                             xg[:, kt, :w],
                                             start=(kt == 0),
                                             stop=(kt == NKT - 1))
                        nc.scalar.activation(
                            k_sb[:, hh, t0 * P:t1 * P], pk[:, :w],
                            AF.Identity, bias=bk_sb[:, hh:hh + 1])
                        for t in range(t0, t1):
                            pv = apv.tile([P, P], F32, tag="pv")
                            for kt in range(NKT):
                                nc.tensor.matmul(
                                    pv[:],
                                    xg[:, kt, (t - t0) * P:(t - t0 + 1) * P],
                                    wv_sb[:, hh, kt],
                                    start=(kt == 0), stop=(kt == NKT - 1))
                            nc.vector.scalar_tensor_tensor(
                                v_sb[:, hh, t, :], pv[:], 1.0,
                                bvb_sb[:, hh * P:(hh + 1) * P],
                                op0=OP.mult, op1=OP.add)

                def emit_epilogue(h, j, po, pr):
                    dst = ot_sb[:, h, j * QCH:(j + 1) * QCH]
                    if j in JD:
                        flag = bt.tile([P, QCH], F32, tag="flag")
                        nc.vector.tensor_scalar(flag[:], pr[:], 0.0, None,
                                                op0=OP.is_equal)
                        rs2 = bt.tile([P, QCH], F32, tag="rs2")
                        nc.vector.tensor_tensor(rs2[:], pr[:], flag[:],
                                                op=OP.add)
                        recip = bt.tile([P, QCH], F32, tag="recip")
                        nc.vector.reciprocal(recip[:], rs2[:])
                        o1 = bt.tile([P, QCH], F32, tag="o1")
                        nc.vector.tensor_tensor(o1[:], po[:], recip[:],
                                                op=OP.mult)
                        nc.vector.scalar_tensor_tensor(
                            dst, flag[:], vm_sb[:, h:h + 1], o1[:],
                            op0=OP.mult, op1=OP.add)
                    else:
                        recip = bt.tile([P, QCH], F32, tag="recip")
                        nc.vector.reciprocal(recip[:], pr[:])
                        nc.vector.tensor_tensor(dst, po[:], recip[:],
                                                op=OP.mult)

                pend = []

                def flush_one():
                    (po, pr, h, kt, ev, first, last, epi) = pend.pop(0)
                    nc.tensor.matmul(po[:], v_sb[:, h, kt, :], ev[:],
                                     start=first, stop=last)
                    nc.tensor.matmul(pr[:], ones_sb[:], ev[:],
                                     start=first, stop=last)
                    if epi is not None:
                        emit_epilogue(*epi)

                def emit_b(h):
                    am_t = xg_tiles[(h, "am")]
                    for j in range(NQCH):
                        tj = T[j]
                        if tj == 0:
                            continue
                        po = bpo.tile([P, QCH], F32, tag="po")
                        pr = bpr.tile([P, QCH], F32, tag="pr")
                        for kt in range(tj):
                            ps = bps.tile([P, QCH], F32, tag="ps")
                            nc.tensor.matmul(
                                ps[:], k_sb[:, h, kt * P:(kt + 1) * P],
                                q_sb[:, h, j * QCH:(j + 1) * QCH],
                                start=True, stop=True)
                            while len(pend) >= 2:
                                flush_one()
                            ev = bev.tile([P, QCH], BF16, tag="ev")
                            nc.scalar.activation(
                                ev[:], ps[:], AF.Exp,
                                bias=tpa_sb[:, h, kt:kt + 1],
                                scale=INV_SQRT_DH)
                            if (j, kt) in eidx:
                                nc.vector.tensor_tensor(
                                    ev[:], ev[:], am_t[:, eidx[(j, kt)], :],
                                    op=OP.mult)
                            epi = (h, j, po, pr) if kt == tj - 1 else None
                            pend.append((po, pr, h, kt, ev,
                                         kt == 0, kt == tj - 1, epi))

                emit_a2(0)
                emit_a2(1)
                emit_b(0)
                emit_a2(2)
                emit_b(1)
                emit_a2(3)
                emit_b(2)
                emit_b(3)
                while pend:
                    flush_one()

        # ---------------- Stage C: o_proj partial ----------------
        with tc.tile_pool(name="ccp", bufs=3) as ccp, \
             tc.tile_pool(name="cps", bufs=4, space="PSUM") as cps:
            for st in range(S // P):
                for ec in range(D // QCH):
                    pc = cps.tile([P, QCH], F32, tag="pc")
                    for h in range(HPC):
                        nc.tensor.matmul(
                            pc[:], ot_sb[:, h, st * P:(st + 1) * P],
                            wo_sb[:, h, ec * QCH:(ec + 1) * QCH],
                            start=(h == 0), stop=(h == HPC - 1))
                    osb = ccp.tile([P, QCH], BF16, tag="osb")
                    nc.scalar.activation(osb[:], pc[:], AF.Identity)
                    (nc.sync if ec % 2 == 0 else nc.gpsimd).dma_start(
                        out_d[st * P:(st + 1) * P,
                              ec * QCH:(ec + 1) * QCH], osb[:])

        amp_cm.__exit__(None, None, None)
        pxg_cm.__exit__(None, None, None)
        pre_cm.__exit__(None, None, None)

    _fix_waits(nc, 1)
    return nc


def _host_mask_and_vmean(hidden_states, Wv, bv, Wdt, bdt, A, ratio_permille):
    """Dynamic-mask pipeline on host, bit-matched to the jax reference."""
    import jax
    import jax.numpy as jnp

    cpu = jax.devices("cpu")[0]
    with jax.default_device(cpu):
        hs = jnp.asarray(hidden_states, dtype=jnp.float32)
        v_lin = jnp.einsum('bsd,ed->bse', hs, jnp.asarray(Wv, jnp.float32)) \
            + jnp.asarray(bv, jnp.float32)
        dt = jnp.einsum('bsd,hd->bsh', v_lin, jnp.asarray(Wdt, jnp.float32)) \
            + jnp.asarray(bdt, jnp.float32)
        dyn = jnp.exp(jnp.asarray(A, jnp.float32) * jax.nn.softplus(dt))
        dynT = dyn.transpose(0, 2, 1)                       # [B, H, S]
        ratio = float(ratio_permille) / 1000.0
        num = int(S * ratio)
        if 0.0 < ratio < 1.0 and num > 0:
            kth = jnp.sort(dynT, axis=-1)[..., num - 1:num]
            tmask = jnp.where(dynT < kth, NEG, dynT)
        else:
            tmask = dynT
        vmean = v_lin.mean(axis=1)                          # [B, D]
        tmask = np.asarray(tmask, dtype=np.float32)
        vmean = np.asarray(vmean, dtype=np.float32)
    return np.maximum(tmask, np.float32(NEG)), vmean


def kernel(hidden_states, attention_mask, Wq, bq, Wk, bk, Wv, bv,
           Wdt, bdt, A, Wo, bo, ratio_permille):
    f32 = np.float32
    hidden_states = np.asarray(hidden_states, f32)
    attention_mask = np.asarray(attention_mask, f32)
    Wq, bq = np.asarray(Wq, f32), np.asarray(bq, f32)
    Wk, bk = np.asarray(Wk, f32), np.asarray(bk, f32)
    Wv, bv = np.asarray(Wv, f32), np.asarray(bv, f32)
    Wdt, bdt = np.asarray(Wdt, f32), np.asarray(bdt, f32)
    A_, Wo, bo = np.asarray(A, f32), np.asarray(Wo, f32), np.asarray(bo, f32)

    tmask, vmean = _host_mask_and_vmean(hidden_states, Wv, bv, Wdt, bdt, A_,
                                        ratio_permille)
    okb = attention_mask[:, 0] != np.float32(MIN32)         # [B, S, S] (q, k)

    # ---- shared program structure from the actual data ----
    survs = {}
    for b in range(B):
        for h in range(H):
            survs[(b, h)] = np.nonzero(tmask[b, h] > NEGT)[0]

    Tj = np.zeros(NQCH, np.int64)
    deg = np.zeros(NQCH, bool)
    for b in range(B):
        for h in range(H):
            sv = survs[(b, h)]
            okr = okb[b][:, sv] if sv.size else np.zeros((S, 0), bool)
            for j in range(NQCH):
                sub = okr[j * QCH:(j + 1) * QCH]
                anyv = sub.any(axis=0)
                nz = np.nonzero(anyv)[0]
                tc_ = 0 if nz.size == 0 else int(nz[-1]) // P + 1
                Tj[j] = max(Tj[j], tc_)
                if sub.shape[1] == 0 or not sub.any(axis=1).all():
                    deg[j] = True
    T = tuple(int(t) for t in Tj)
    NT = max(max(T), 1)
    KP = NT * P

    okg_pads = {}
    edge_need = [set() for _ in range(NQCH)]
    for b in range(B):
        for h in range(H):
            sv = survs[(b, h)]
            ns = sv.size
            svp = np.concatenate(
                [sv, np.full(KP - ns, sv[-1] if ns else 0, sv.dtype)])
            okg = np.ones((S, KP), bool)
            if ns:
                okg[:, :ns] = okb[b][:, sv]
            else:
                okg[:] = True
            okg_pads[(b, h)] = (svp, ns, okg)
            for j in range(NQCH):
                for kt in range(T[j]):
                    if not okg[j * QCH:(j + 1) * QCH, kt * P:(kt + 1) * P].all():
                        edge_need[j].add(kt)
    EDGE = tuple(tuple(sorted(e)) for e in edge_need)
    JD = tuple(int(j) for j in range(NQCH) if deg[j])
    edge_list = [(j, kt) for j in range(NQCH) for kt in EDGE[j]]
    NE = max(len(edge_list), 1)

    cfg = {"T": T, "NT": NT, "EDGE": EDGE, "JD": JD}
    key = (T, NT, EDGE, JD)
    if _prog_cache.get("key") != key:
        _prog_cache["nc"] = _build_program(cfg)
        _prog_cache["key"] = key
    nc = _prog_cache["nc"]

    ones_blk = np.ones((P, P), NPBF16)
    in_maps = []
    for c in range(NCORE):
        b, hg = divmod(c, HGRP)
        h0 = hg * HPC
        e0 = hg * HD
        x = hidden_states[b]                                 # [S, D]

        xt = np.ascontiguousarray(
            x.reshape(NXC, XCH, NKT, P).transpose(0, 3, 2, 1)).astype(NPBF16)
        xkg = np.empty((HPC, P, NKT, KP), NPBF16)
        tpa = np.empty((P, HPC, NT), f32)
        am_c = np.zeros((HPC, P, NE, QCH), NPBF16)
        for hh in range(HPC):
            h = h0 + hh
            svp, ns, okg = okg_pads[(b, h)]
            xg = x[svp]                                      # [KP, D]
            xkg[hh] = xg.reshape(KP, NKT, P).transpose(2, 1, 0).astype(NPBF16)
            vals = np.full(KP, NEG, f32)
            vals[:ns] = tmask[b, h, svp[:ns]]
            tpa[:, hh, :] = vals.reshape(NT, P).T
            for idx, (j, kt) in enumerate(edge_list):
                am_c[hh, :, idx, :] = \
                    okg[j * QCH:(j + 1) * QCH, kt * P:(kt + 1) * P].T

        def lhsfmt(W):
            t = W[e0:e0 + HD].reshape(HPC, P, NKT, P)
            return np.ascontiguousarray(t.transpose(3, 0, 2, 1)).astype(NPBF16)

        wq_c = lhsfmt(Wq)
        wk_c = lhsfmt(Wk)
        wv_c = lhsfmt(Wv)
        wo_c = np.ascontiguousarray(
            Wo[e0:e0 + HD].reshape(HPC, P, D).transpose(1, 0, 2)).astype(NPBF16)
        bq_c = np.ascontiguousarray(bq[e0:e0 + HD].reshape(HPC, P).T)
        bk_c = np.ascontiguousarray(bk[e0:e0 + HD].reshape(HPC, P).T)
        bvb_c = np.ascontiguousarray(
            np.broadcast_to(bv[e0:e0 + HD], (P, HD))).astype(f32)
        vm_c = np.ascontiguousarray(vmean[b, e0:e0 + HD].reshape(HPC, P).T)

        in_maps.append({
            "xt": xt, "xkg": xkg, "wq": wq_c, "wk": wk_c, "wv": wv_c,
            "wo": wo_c, "am": am_c, "tp": tpa, "onesw": ones_blk,
            "bqv": bq_c, "bkv": bk_c, "bvb": bvb_c, "vm": vm_c,
        })

    res = run_bass_kernel_spmd(nc, in_maps, list(range(NCORE)))

    out = np.zeros((B, S, D), np.float64)
    for c in range(NCORE):
        b = c // HGRP
        out[b] += res.results[c]["opart"].astype(np.float64)
    out += bo.astype(np.float64)
    return out.astype(f32)


# revision 4
# speedup vs baseline: 1.2949x; 1.0127x over previous
"""Trainium2 Bass kernel for sparse-attention (nn_DMA_14903536517676).

Full (unsharded) inputs in, full output out. Shards across 8 NeuronCores:
data-parallel over batch (B=2) x tensor-parallel over heads (4 per core).

Key idea vs the dense baseline: the dynamic (kth-threshold) mask depends only
on the KEY position, killing ~half of all keys per (batch, head). The host
computes the surviving-key set per head and gathers the corresponding x
columns; the device then computes K/V projections and the whole attention
block only over compacted survivor slots (causal q-chunks touch only a
prefix of the survivor list). All matmul operands are bf16 (full PE rate at
any tile width in fp32-accumulate), Q/K/V stay SBUF-resident (no DRAM
scratch roundtrip), and the o_proj partial is written bf16 and reduced on
host. Dynamic-mask values ride the exp bias per survivor slot; the ragged
causal edge is a small set of host-built 0/1 tiles multiplied into ev.
"""

import math

import numpy as np
import ml_dtypes

import concourse.bass as bass
import concourse.mybir as mybir
import concourse.tile as tile
from concourse.bass_utils import run_bass_kernel_spmd

F32 = mybir.dt.float32
F32R = mybir.dt.float32r
BF16 = mybir.dt.bfloat16
NPBF16 = ml_dtypes.bfloat16
AF = mybir.ActivationFunctionType
OP = mybir.AluOpType

B, S, D, H = 2, 2048, 2048, 16
P = 128
DH = D // H            # 128
NCORE = 8
HGRP = NCORE // B      # 4 head-groups (cores) per batch element
HPC = H // HGRP        # 4 heads per core
HD = HPC * DH          # 512 head dims per core
NKT = D // P           # 16 contraction tiles over D
XCH = 256              # Q-projection s-chunk width
NXC = S // XCH         # 8
QCH = 512              # attention q-chunk width
NQCH = S // QCH        # 4
MIN32 = float(np.finfo(np.float32).min)
NEG = -1.0e37          # clamped mask sentinel (exp underflows to exact 0)
NEGT = -1.0e36         # threshold for "is masked" tests on host
INV_SQRT_DH = 1.0 / math.sqrt(DH)

_prog_cache = {}


# ---------------------------------------------------------------------------
# Walrus in this toolchain accepts at most ONE embedded sync-wait command per
# instruction. Tile emits more. Move overflow waits onto InstNoOp
# instructions inserted immediately before, on the same engine (semaphores
# are monotonic in this loop-free program, so waiting earlier is safe).
# ---------------------------------------------------------------------------
def _fix_waits(nc, maxw=1):
    uid = 0
    for f in nc.m.functions:
        for b in f.blocks:
            il = b.instructions
            idx = 0
            while idx < len(il):
                inst = il[idx]
                si = getattr(inst, "sync_info", None)
                if si is None:
                    idx += 1
                    continue
                waits = si.on_wait or []
                if len(waits) <= maxw:
                    idx += 1
                    continue
                si.on_wait = waits[-maxw:]
                overflow = waits[:-maxw]
                nops = []
                for j in range(0, len(overflow), maxw):
                    uid += 1
                    nop = mybir.InstNoOp(name=f"I-waitnop-{uid}")
                    nop.engine = inst.engine
                    nop.sync_info = mybir.SyncInfo(
                        on_wait=overflow[j:j + maxw], on_update=[])
                    nops.append(nop)
                for k, nop in enumerate(nops):
                    il.insert(idx + k, nop)
                idx += len(nops) + 1


def _chunk_plan(nt):
    """Split nt 128-wide tiles into DMA/compute chunks of 2 (last 3 if odd)."""
    if nt <= 3:
        return [(0, nt)]
    cuts = list(range(0, nt - 3, 2)) + [nt - 3 if nt % 2 else nt - 2]
    cuts = sorted(set(c for c in cuts if c >= 0))
    plan = []
    prev = 0
    for c in cuts[1:] + [nt]:
        plan.append((prev, c))
        prev = c
    return plan


def _build_program(cfg):
    T = cfg["T"]
    NT = cfg["NT"]
    KP = NT * P
    EDGE = cfg["EDGE"]
    JD = set(cfg["JD"])
    edge_list = [(j, kt) for j in range(NQCH) for kt in EDGE[j]]
    NE = max(len(edge_list), 1)
    eidx = {jk: i for i, jk in enumerate(edge_list)}

    nc = bass.Bass()

    xt_d = nc.declare_dram_parameter("xt", [NXC, P, NKT, XCH], BF16, isOutput=False)
    xkg_d = nc.declare_dram_parameter("xkg", [HPC, P, NKT, KP], BF16, isOutput=False)
    wq_d = nc.declare_dram_parameter("wq", [P, HPC, NKT, P], BF16, isOutput=False)
    wk_d = nc.declare_dram_parameter("wk", [P, HPC, NKT, P], BF16, isOutput=False)
    wv_d = nc.declare_dram_parameter("wv", [P, HPC, NKT, P], BF16, isOutput=False)
    wo_d = nc.declare_dram_parameter("wo", [P, HPC, D], BF16, isOutput=False)
    am_d = nc.declare_dram_parameter("am", [HPC, P, NE, QCH], BF16, isOutput=False)
    tpa_d = nc.declare_dram_parameter("tp", [P, HPC, NT], F32, isOutput=False)
    ones_d = nc.declare_dram_parameter("onesw", [P, P], BF16, isOutput=False)
    bq_d = nc.declare_dram_parameter("bqv", [P, HPC], F32, isOutput=False)
    bk_d = nc.declare_dram_parameter("bkv", [P, HPC], F32, isOutput=False)
    bvb_d = nc.declare_dram_parameter("bvb", [P, HD], F32, isOutput=False)
    vm_d = nc.declare_dram_parameter("vm", [P, HPC], F32, isOutput=False)
    out_d = nc.declare_dram_parameter("opart", [S, D], BF16, isOutput=True)

    plan = _chunk_plan(NT)
    WMAX = max(t1 - t0 for t0, t1 in plan) * P

    with tile.TileContext(nc) as tc:
        pre_cm = tc.tile_pool(name="pre", bufs=1)
        pre = pre_cm.__enter__()
        q_sb = pre.tile([P, HPC, S], BF16)
        k_sb = pre.tile([P, HPC, KP], BF16)
        v_sb = pre.tile([P, HPC, NT, P], BF16)
        ot_sb = pre.tile([P, HPC, S], BF16)
        wo_sb = pre.tile([P, HPC, D], BF16)
        ones_sb = pre.tile([P, P], BF16)
        tpa_sb = pre.tile([P, HPC, NT], F32)
        vm_sb = pre.tile([P, HPC], F32)
        bq_sb = pre.tile([P, HPC], F32)
        bk_sb = pre.tile([P, HPC], F32)
        bvb_sb = pre.tile([P, HD], F32)
        pxg_cm = tc.tile_pool(name="pxg", bufs=2)
        pxg = pxg_cm.__enter__()
        amp_cm = tc.tile_pool(name="amp", bufs=2)
        amp = amp_cm.__enter__()

        # small constants early on the scalar queue
        nc.scalar.dma_start(tpa_sb[:], tpa_d[:])
        nc.scalar.dma_start(vm_sb[:], vm_d[:])
        nc.scalar.dma_start(bq_sb[:], bq_d[:])
        nc.scalar.dma_start(bk_sb[:], bk_d[:])
        nc.scalar.dma_start(bvb_sb[:], bvb_d[:])
        nc.scalar.dma_start(ones_sb[:], ones_d[:])

        with tc.tile_pool(name="aw", bufs=1) as aw:
            wk_sb = aw.tile([P, HPC, NKT, P], BF16)
            wv_sb = aw.tile([P, HPC, NKT, P], BF16)

            apk_cm = tc.tile_pool(name="apk", bufs=1, space="PSUM")
            apk = apk_cm.__enter__()
            apv_cm = tc.tile_pool(name="apv", bufs=1, space="PSUM")
            apv = apv_cm.__enter__()

            def emit_a2(hh):
                for ci, (t0, t1) in enumerate(plan):
                    w = (t1 - t0) * P
                    xg = xg_tiles[(hh, ci)]
                    pk = apk.tile([P, WMAX], F32, tag="pk")
                    for kt in range(NKT):
                        nc.tensor.matmul(pk[:, :w], wk_sb[:, hh, kt],
                                         xg[:, kt, :w],
                                         start=(kt == 0),
                                         stop=(kt == NKT - 1))
                    nc.scalar.activation(
                        k_sb[:, hh, t0 * P:t1 * P], pk[:, :w],
                        AF.Identity, bias=bk_sb[:, hh:hh + 1])
                    for t in range(t0, t1):
                        pv = apv.tile([P, P], F32, tag="pv")
                        for kt in range(NKT):
                            nc.tensor.matmul(
                                pv[:],
                                xg[:, kt, (t - t0) * P:(t - t0 + 1) * P],
                                wv_sb[:, hh, kt],
                                start=(kt == 0), stop=(kt == NKT - 1))
                        nc.vector.scalar_tensor_tensor(
                            v_sb[:, hh, t, :], pv[:], 1.0,
                            bvb_sb[:, hh * P:(hh + 1) * P],
                            op0=OP.mult, op1=OP.add)

            # ------------- Stage A: Q projection (dense) -------------
            with tc.tile_pool(name="awq", bufs=1) as awq, \
                 tc.tile_pool(name="ax", bufs=2) as ax, \
                 tc.tile_pool(name="apq", bufs=3, space="PSUM") as apq:
                wq_sb = awq.tile([P, HPC, NKT, P], BF16)
                # dep-free PE warmup (ramps the p-state before real work)
                dmy = awq.tile([P, 5 * P], F32R)
                nc.vector.memset(dmy.bitcast(F32)[:], 1.0)
                for _ in range(8):
                    pdmy = apq.tile([P, 4 * P], F32, tag="pq", name="pdmy")
                    nc.tensor.matmul(pdmy[:], dmy[:, :P], dmy[:, P:],
                                     start=True, stop=True)
                nc.scalar.dma_start(wq_sb[:], wq_d[:])
                nc.scalar.dma_start(wk_sb[:], wk_d[:])
                nc.scalar.dma_start(wv_sb[:], wv_d[:])
                # xkg streamed on gpsimd for all heads (chunked); am per head
                xg_tiles = {}
                for hh in range(HPC):
                    for ci, (t0, t1) in enumerate(plan):
                        w = (t1 - t0) * P
                        xg = pxg.tile([P, NKT, WMAX], BF16, tag="xkg")
                        nc.gpsimd.dma_start(
                            xg[:, :, :w], xkg_d[hh][:, :, t0 * P:t1 * P])
                        xg_tiles[(hh, ci)] = xg
                    am_t = amp.tile([P, NE, QCH], BF16, tag="am")
                    nc.gpsimd.dma_start(am_t[:], am_d[hh])
                    xg_tiles[(hh, "am")] = am_t

                for c in range(NXC):
                    if c == 4:
                        emit_a2(0)
                    xt = ax.tile([P, NKT, XCH], BF16, tag="xt")
                    nc.sync.dma_start(xt[:], xt_d[c])
                    for hh in range(HPC):
                        pq = apq.tile([P, XCH], F32, tag="pq", name="pq")
                        for kt in range(NKT):
                            nc.tensor.matmul(pq[:], wq_sb[:, hh, kt],
                                             xt[:, kt],
                                             start=(kt == 0),
                                             stop=(kt == NKT - 1))
                        nc.scalar.activation(
                            q_sb[:, hh, c * XCH:(c + 1) * XCH], pq[:],
                            AF.Identity, bias=bq_sb[:, hh:hh + 1])
            nc.scalar.dma_start(wo_sb[:], wo_d[:])

            # -------- Stage A2 (K/V compact) + Stage B (attention) ---
            with tc.tile_pool(name="bev", bufs=3) as bev, \
                 tc.tile_pool(name="bt", bufs=2) as bt, \
                 tc.tile_pool(name="bps", bufs=3, space="PSUM") as bps, \
                 tc.tile_pool(name="bpo", bufs=2, space="PSUM") as bpo, \
                 tc.tile_pool(name="bpr", bufs=1, space="PSUM") as bpr:

                def emit_epilogue(h, j, po, pr):
                    dst = ot_sb[:, h, j * QCH:(j + 1) * QCH]
                    if j in JD:
                        flag = bt.tile([P, QCH], F32, tag="flag")
                        nc.vector.tensor_scalar(flag[:], pr[:], 0.0, None,
                                                op0=OP.is_equal)
                        rs2 = bt.tile([P, QCH], F32, tag="rs2")
                        nc.vector.tensor_tensor(rs2[:], pr[:], flag[:],
                                                op=OP.add)
                        recip = bt.tile([P, QCH], F32, tag="recip")
                        nc.vector.reciprocal(recip[:], rs2[:])
                        o1 = bt.tile([P, QCH], F32, tag="o1")
                        nc.vector.tensor_tensor(o1[:], po[:], recip[:],
                                                op=OP.mult)
                        nc.vector.scalar_tensor_tensor(
                            dst, flag[:], vm_sb[:, h:h + 1], o1[:],
                            op0=OP.mult, op1=OP.add)
                    else:
                        recip = bt.tile([P, QCH], F32, tag="recip")
                        nc.vector.reciprocal(recip[:], pr[:])
                        nc.vector.tensor_tensor(dst, po[:], recip[:],
                                                op=OP.mult)

                pend = []

                def flush_one():
                    (po, pr, h, kt, ev, first, last, epi) = pend.pop(0)
                    nc.tensor.matmul(po[:], v_sb[:, h, kt, :], ev[:],
                                     start=first, stop=last)
                    nc.tensor.matmul(pr[:], ones_sb[:], ev[:],
                                     start=first, stop=last)
                    if epi is not None:
                        emit_epilogue(*epi)

                def emit_b(h):
                    am_t = xg_tiles[(h, "am")]
                    for j in range(NQCH):
                        tj = T[j]
                        if tj == 0:
                            continue
                        po = bpo.tile([P, QCH], F32, tag="po")
                        pr = bpr.tile([P, QCH], F32, tag="pr")
                        for kt in range(tj):
                            ps = bps.tile([P, QCH], F32, tag="ps")
                            nc.tensor.matmul(
                                ps[:], k_sb[:, h, kt * P:(kt + 1) * P],
                                q_sb[:, h, j * QCH:(j + 1) * QCH],
                                start=True, stop=True)
                            while len(pend) >= 2:
                                flush_one()
                            ev = bev.tile([P, QCH], BF16, tag="ev")
                            nc.scalar.activation(
                                ev[:], ps[:], AF.Exp,
                                bias=tpa_sb[:, h, kt:kt + 1],
                                scale=INV_SQRT_DH)
                            if (j, kt) in eidx:
                                nc.vector.tensor_tensor(
                                    ev[:], ev[:], am_t[:, eidx[(j, kt)], :],
                                    op=OP.mult)
                            epi = (h, j, po, pr) if kt == tj - 1 else None
                            pend.append((po, pr, h, kt, ev,
                                         kt == 0, kt == tj - 1, epi))

                emit_a2(1)
                emit_b(0)
                emit_a2(2)
                emit_b(1)
                emit_a2(3)
                emit_b(2)
                emit_b(3)
                while pend:
                    flush_one()
            apv_cm.__exit__(None, None, None)
            apk_cm.__exit__(None, None, None)

        # ---------------- Stage C: o_proj partial ----------------
        with tc.tile_pool(name="ccp", bufs=3) as ccp, \
             tc.tile_pool(name="cps", bufs=4, space="PSUM") as cps:
            for st in range(S // P):
                for ec in range(D // QCH):
                    pc = cps.tile([P, QCH], F32, tag="pc")
                    for h in range(HPC):
                        nc.tensor.matmul(
                            pc[:], ot_sb[:, h, st * P:(st + 1) * P],
                            wo_sb[:, h, ec * QCH:(ec + 1) * QCH],
                            start=(h == 0), stop=(h == HPC - 1))
                    osb = ccp.tile([P, QCH], BF16, tag="osb")
                    nc.scalar.activation(osb[:], pc[:], AF.Identity)
                    (nc.sync if ec % 2 == 0 else nc.gpsimd).dma_start(
                        out_d[st * P:(st + 1) * P,
                              ec * QCH:(ec + 1) * QCH], osb[:])

        amp_cm.__exit__(None, None, None)
        pxg_cm.__exit__(None, None, None)
        pre_cm.__exit__(None, None, None)

    _fix_waits(nc, 1)
    return nc


def _host_mask_and_vmean(hidden_states, Wv, bv, Wdt, bdt, A, ratio_permille):
    """Dynamic-mask pipeline on host, bit-matched to the jax reference."""
    import jax
    import jax.numpy as jnp

    cpu = jax.devices("cpu")[0]
    with jax.default_device(cpu):
        hs = jnp.asarray(hidden_states, dtype=jnp.float32)
        v_lin = jnp.einsum('bsd,ed->bse', hs, jnp.asarray(Wv, jnp.float32)) \
            + jnp.asarray(bv, jnp.float32)
        dt = jnp.einsum('bsd,hd->bsh', v_lin, jnp.asarray(Wdt, jnp.float32)) \
            + jnp.asarray(bdt, jnp.float32)
        dyn = jnp.exp(jnp.asarray(A, jnp.float32) * jax.nn.softplus(dt))
        dynT = dyn.transpose(0, 2, 1)                       # [B, H, S]
        ratio = float(ratio_permille) / 1000.0
        num = int(S * ratio)
        if 0.0 < ratio < 1.0 and num > 0:
            kth = jnp.sort(dynT, axis=-1)[..., num - 1:num]
            tmask = jnp.where(dynT < kth, NEG, dynT)
        else:
            tmask = dynT
        vmean = v_lin.mean(axis=1)                          # [B, D]
        tmask = np.asarray(tmask, dtype=np.float32)
        vmean = np.asarray(vmean, dtype=np.float32)
    return np.maximum(tmask, np.float32(NEG)), vmean


def kernel(hidden_states, attention_mask, Wq, bq, Wk, bk, Wv, bv,
           Wdt, bdt, A, Wo, bo, ratio_permille):
    f32 = np.float32
    hidden_states = np.asarray(hidden_states, f32)
    attention_mask = np.asarray(attention_mask, f32)
    Wq, bq = np.asarray(Wq, f32), np.asarray(bq, f32)
    Wk, bk = np.asarray(Wk, f32), np.asarray(bk, f32)
    Wv, bv = np.asarray(Wv, f32), np.asarray(bv, f32)
    Wdt, bdt = np.asarray(Wdt, f32), np.asarray(bdt, f32)
    A_, Wo, bo = np.asarray(A, f32), np.asarray(Wo, f32), np.asarray(bo, f32)

    tmask, vmean = _host_mask_and_vmean(hidden_states, Wv, bv, Wdt, bdt, A_,
                                        ratio_permille)
    okb = attention_mask[:, 0] != np.float32(MIN32)         # [B, S, S] (q, k)

    # ---- shared program structure from the actual data ----
    survs = {}
    for b in range(B):
        for h in range(H):
            survs[(b, h)] = np.nonzero(tmask[b, h] > NEGT)[0]

    Tj = np.zeros(NQCH, np.int64)
    deg = np.zeros(NQCH, bool)
    for b in range(B):
        for h in range(H):
            sv = survs[(b, h)]
            okr = okb[b][:, sv] if sv.size else np.zeros((S, 0), bool)
            for j in range(NQCH):
                sub = okr[j * QCH:(j + 1) * QCH]
                anyv = sub.any(axis=0)
                nz = np.nonzero(anyv)[0]
                tc_ = 0 if nz.size == 0 else int(nz[-1]) // P + 1
                Tj[j] = max(Tj[j], tc_)
                if sub.shape[1] == 0 or not sub.any(axis=1).all():
                    deg[j] = True
    T = tuple(int(t) for t in Tj)
    NT = max(max(T), 1)
    KP = NT * P

    okg_pads = {}
    edge_need = [set() for _ in range(NQCH)]
    for b in range(B):
        for h in range(H):
            sv = survs[(b, h)]
            ns = sv.size
            svp = np.concatenate(
                [sv, np.full(KP - ns, sv[-1] if ns else 0, sv.dtype)])
            okg = np.ones((S, KP), bool)
            if ns:
                okg[:, :ns] = okb[b][:, sv]
            else:
                okg[:] = True
            okg_pads[(b, h)] = (svp, ns, okg)
            for j in range(NQCH):
                for kt in range(T[j]):
                    if not okg[j * QCH:(j + 1) * QCH, kt * P:(kt + 1) * P].all():
                        edge_need[j].add(kt)
    EDGE = tuple(tuple(sorted(e)) for e in edge_need)
    JD = tuple(int(j) for j in range(NQCH) if deg[j])
    edge_list = [(j, kt) for j in range(NQCH) for kt in EDGE[j]]
    NE = max(len(edge_list), 1)

    cfg = {"T": T, "NT": NT, "EDGE": EDGE, "JD": JD}
    key = (T, NT, EDGE, JD)
    if _prog_cache.get("key") != key:
        _prog_cache["nc"] = _build_program(cfg)
        _prog_cache["key"] = key
    nc = _prog_cache["nc"]

    ones_blk = np.ones((P, P), NPBF16)
    in_maps = []
    for c in range(NCORE):
        b, hg = divmod(c, HGRP)
        h0 = hg * HPC
        e0 = hg * HD
        x = hidden_states[b]                                 # [S, D]

        xt = np.ascontiguousarray(
            x.reshape(NXC, XCH, NKT, P).transpose(0, 3, 2, 1)).astype(NPBF16)
        xkg = np.empty((HPC, P, NKT, KP), NPBF16)
        tpa = np.empty((P, HPC, NT), f32)
        am_c = np.zeros((HPC, P, NE, QCH), NPBF16)
        for hh in range(HPC):
            h = h0 + hh
            svp, ns, okg = okg_pads[(b, h)]
            xg = x[svp]                                      # [KP, D]
            xkg[hh] = xg.reshape(KP, NKT, P).transpose(2, 1, 0).astype(NPBF16)
            vals = np.full(KP, NEG, f32)
            vals[:ns] = tmask[b, h, svp[:ns]]
            tpa[:, hh, :] = vals.reshape(NT, P).T
            for idx, (j, kt) in enumerate(edge_list):
                am_c[hh, :, idx, :] = \
                    okg[j * QCH:(j + 1) * QCH, kt * P:(kt + 1) * P].T

        def lhsfmt(W):
            t = W[e0:e0 + HD].reshape(HPC, P, NKT, P)
            return np.ascontiguousarray(t.transpose(3, 0, 2, 1)).astype(NPBF16)

        wq_c = lhsfmt(Wq)
        wk_c = lhsfmt(Wk)
        wv_c = lhsfmt(Wv)
        wo_c = np.ascontiguousarray(
            Wo[e0:e0 + HD].reshape(HPC, P, D).transpose(1, 0, 2)).astype(NPBF16)
        bq_c = np.ascontiguousarray(bq[e0:e0 + HD].reshape(HPC, P).T)
        bk_c = np.ascontiguousarray(bk[e0:e0 + HD].reshape(HPC, P).T)
        bvb_c = np.ascontiguousarray(
            np.broadcast_to(bv[e0:e0 + HD], (P, HD))).astype(f32)
        vm_c = np.ascontiguousarray(vmean[b, e0:e0 + HD].reshape(HPC, P).T)

        in_maps.append({
            "xt": xt, "xkg": xkg, "wq": wq_c, "wk": wk_c, "wv": wv_c,
            "wo": wo_c, "am": am_c, "tp": tpa, "onesw": ones_blk,
            "bqv": bq_c, "bkv": bk_c, "bvb": bvb_c, "vm": vm_c,
        })

    res = run_bass_kernel_spmd(nc, in_maps, list(range(NCORE)))

    out = np.zeros((B, S, D), np.float64)
    for c in range(NCORE):
        b = c // HGRP
        out[b] += res.results[c]["opart"].astype(np.float64)
    out += bo.astype(np.float64)
    return out.astype(f32)


# revision 7
# speedup vs baseline: 1.3058x; 1.0084x over previous
"""Trainium2 Bass kernel for sparse-attention (nn_DMA_14903536517676).

Full (unsharded) inputs in, full output out. Shards across 8 NeuronCores:
data-parallel over batch (B=2) x tensor-parallel over heads (4 per core).

Key idea vs the dense baseline: the dynamic (kth-threshold) mask depends only
on the KEY position, killing ~half of all keys per (batch, head). The host
computes the surviving-key set per head and gathers the corresponding x
columns; the device then computes K/V projections and the whole attention
block only over compacted survivor slots (causal q-chunks touch only a
prefix of the survivor list). All matmul operands are bf16 (full PE rate at
any tile width in fp32-accumulate), Q/K/V stay SBUF-resident (no DRAM
scratch roundtrip), and the o_proj partial is written bf16 and reduced on
host. Dynamic-mask values ride the exp bias per survivor slot; the ragged
causal edge is a small set of host-built 0/1 tiles multiplied into ev.
"""

import math

import numpy as np
import ml_dtypes

import concourse.bass as bass
import concourse.mybir as mybir
import concourse.tile as tile
from concourse.bass_utils import run_bass_kernel_spmd

F32 = mybir.dt.float32
F32R = mybir.dt.float32r
BF16 = mybir.dt.bfloat16
NPBF16 = ml_dtypes.bfloat16
AF = mybir.ActivationFunctionType
OP = mybir.AluOpType

B, S, D, H = 2, 2048, 2048, 16
P = 128
DH = D // H            # 128
NCORE = 8
HGRP = NCORE // B      # 4 head-groups (cores) per batch element
HPC = H // HGRP        # 4 heads per core
HD = HPC * DH          # 512 head dims per core
NKT = D // P           # 16 contraction tiles over D
XCH = 256              # Q-projection s-chunk width
NXC = S // XCH         # 8
QCH = 512              # attention q-chunk width
NQCH = S // QCH        # 4
MIN32 = float(np.finfo(np.float32).min)
NEG = -1.0e37          # clamped mask sentinel (exp underflows to exact 0)
NEGT = -1.0e36         # threshold for "is masked" tests on host
INV_SQRT_DH = 1.0 / math.sqrt(DH)

_prog_cache = {}


# ---------------------------------------------------------------------------
# Walrus in this toolchain accepts at most ONE embedded sync-wait command per
# instruction. Tile emits more. Move overflow waits onto InstNoOp
# instructions inserted immediately before, on the same engine (semaphores
# are monotonic in this loop-free program, so waiting earlier is safe).
# ---------------------------------------------------------------------------
def _fix_waits(nc, maxw=1):
    uid = 0
    for f in nc.m.functions:
        for b in f.blocks:
            il = b.instructions
            idx = 0
            while idx < len(il):
                inst = il[idx]
                si = getattr(inst, "sync_info", None)
                if si is None:
                    idx += 1
                    continue
                waits = si.on_wait or []
                if len(waits) <= maxw:
                    idx += 1
                    continue
                si.on_wait = waits[-maxw:]
                overflow = waits[:-maxw]
                nops = []
                for j in range(0, len(overflow), maxw):
                    uid += 1
                    nop = mybir.InstNoOp(name=f"I-waitnop-{uid}")
                    nop.engine = inst.engine
                    nop.sync_info = mybir.SyncInfo(
                        on_wait=overflow[j:j + maxw], on_update=[])
                    nops.append(nop)
                for k, nop in enumerate(nops):
                    il.insert(idx + k, nop)
                idx += len(nops) + 1


def _chunk_plan(nt):
    """Split nt 128-wide tiles into DMA/compute chunks of 2 (last 3 if odd)."""
    if nt <= 3:
        return [(0, nt)]
    cuts = list(range(0, nt - 3, 2)) + [nt - 3 if nt % 2 else nt - 2]
    cuts = sorted(set(c for c in cuts if c >= 0))
    plan = []
    prev = 0
    for c in cuts[1:] + [nt]:
        plan.append((prev, c))
        prev = c
    return plan


def _build_program(cfg):
    T = cfg["T"]
    NT = cfg["NT"]
    KP = NT * P
    EDGE = cfg["EDGE"]
    JD = set(cfg["JD"])
    edge_list = [(j, kt) for j in range(NQCH) for kt in EDGE[j]]
    NE = max(len(edge_list), 1)
    eidx = {jk: i for i, jk in enumerate(edge_list)}

    nc = bass.Bass()

    xt_d = nc.declare_dram_parameter("xt", [NXC, P, NKT, XCH], BF16, isOutput=False)
    xkg_d = nc.declare_dram_parameter("xkg", [HPC, P, NKT, KP], BF16, isOutput=False)
    wq_d = nc.declare_dram_parameter("wq", [P, HPC, NKT, P], BF16, isOutput=False)
    wk_d = nc.declare_dram_parameter("wk", [P, HPC, NKT, P], BF16, isOutput=False)
    wv_d = nc.declare_dram_parameter("wv", [P, HPC, NKT, P], BF16, isOutput=False)
    wo_d = nc.declare_dram_parameter("wo", [P, HPC, D], BF16, isOutput=False)
    am_d = nc.declare_dram_parameter("am", [HPC, P, NE, QCH], BF16, isOutput=False)
    tpa_d = nc.declare_dram_parameter("tp", [P, HPC, NT], F32, isOutput=False)
    ones_d = nc.declare_dram_parameter("onesw", [P, P], BF16, isOutput=False)
    bq_d = nc.declare_dram_parameter("bqv", [P, HPC], F32, isOutput=False)
    bk_d = nc.declare_dram_parameter("bkv", [P, HPC], F32, isOutput=False)
    bvb_d = nc.declare_dram_parameter("bvb", [P, HD], F32, isOutput=False)
    vm_d = nc.declare_dram_parameter("vm", [P, HPC], F32, isOutput=False)
    out_d = nc.declare_dram_parameter("opart", [S, D], BF16, isOutput=True)
    dbg = cfg.get("dbg")
    if dbg:
        qd_d = nc.declare_dram_parameter("qdump", [P, HPC, S], BF16, isOutput=True)
        kd_d = nc.declare_dram_parameter("kdump", [P, HPC, KP], BF16, isOutput=True)
        vd_d = nc.declare_dram_parameter("vdump", [P, HPC, NT, P], BF16, isOutput=True)
        od_d = nc.declare_dram_parameter("otdump", [P, HPC, S], BF16, isOutput=True)

    plan = _chunk_plan(NT)
    WMAX = max(t1 - t0 for t0, t1 in plan) * P

    with tile.TileContext(nc) as tc:
        pre_cm = tc.tile_pool(name="pre", bufs=1)
        pre = pre_cm.__enter__()
        q_sb = pre.tile([P, HPC, S], BF16)
        k_sb = pre.tile([P, HPC, KP], BF16)
        v_sb = pre.tile([P, HPC, NT, P], BF16)
        ot_sb = pre.tile([P, HPC, S], BF16)
        wo_sb = pre.tile([P, HPC, D], BF16)
        ones_sb = pre.tile([P, P], BF16)
        tpa_sb = pre.tile([P, HPC, NT], F32)
        vm_sb = pre.tile([P, HPC], F32)
        bq_sb = pre.tile([P, HPC], F32)
        bk_sb = pre.tile([P, HPC], F32)
        bvb_sb = pre.tile([P, HD], F32)
        pxg_cm = tc.tile_pool(name="pxg", bufs=2)
        pxg = pxg_cm.__enter__()
        amp_cm = tc.tile_pool(name="amp", bufs=2)
        amp = amp_cm.__enter__()

        # small constants early on the scalar queue
        nc.scalar.dma_start(tpa_sb[:], tpa_d[:])
        nc.scalar.dma_start(vm_sb[:], vm_d[:])
        nc.scalar.dma_start(bq_sb[:], bq_d[:])
        nc.scalar.dma_start(bk_sb[:], bk_d[:])
        nc.scalar.dma_start(bvb_sb[:], bvb_d[:])
        nc.scalar.dma_start(ones_sb[:], ones_d[:])

        with tc.tile_pool(name="aw", bufs=1) as aw:
            wk_sb = aw.tile([P, HPC, NKT, P], BF16)
            wv_sb = aw.tile([P, HPC, NKT, P], BF16)

            apk_cm = tc.tile_pool(name="apk", bufs=1, space="PSUM")
            apk = apk_cm.__enter__()
            apv_cm = tc.tile_pool(name="apv", bufs=1, space="PSUM")
            apv = apv_cm.__enter__()

            def emit_a2(hh):
                for ci, (t0, t1) in enumerate(plan):
                    w = (t1 - t0) * P
                    xg = xg_tiles[(hh, ci)]
                    pk = apk.tile([P, WMAX], F32, tag="pk")
                    for kt in range(NKT):
                        nc.tensor.matmul(pk[:, :w], wk_sb[:, hh, kt],
                                         xg[:, kt, :w],
                                         start=(kt == 0),
                                         stop=(kt == NKT - 1))
                    nc.scalar.activation(
                        k_sb[:, hh, t0 * P:t1 * P], pk[:, :w],
                        AF.Identity, bias=bk_sb[:, hh:hh + 1])
                    for t in range(t0, t1):
                        pv = apv.tile([P, P], F32, tag="pv")
                        for kt in range(NKT):
                            nc.tensor.matmul(
                                pv[:],
                                xg[:, kt, (t - t0) * P:(t - t0 + 1) * P],
                                wv_sb[:, hh, kt],
                                start=(kt == 0), stop=(kt == NKT - 1))
                        nc.vector.scalar_tensor_tensor(
                            v_sb[:, hh, t, :], pv[:], 1.0,
                            bvb_sb[:, hh * P:(hh + 1) * P],
                            op0=OP.mult, op1=OP.add)

            # ------------- Stage A: Q projection (dense) -------------
            with tc.tile_pool(name="awq", bufs=1) as awq, \
                 tc.tile_pool(name="ax", bufs=2) as ax, \
                 tc.tile_pool(name="apq", bufs=3, space="PSUM") as apq:
                wq_sb = awq.tile([P, HPC, NKT, P], BF16)
                # dep-free PE warmup (ramps the p-state before real work)
                dmy = awq.tile([P, 5 * P], F32R)
                nc.vector.memset(dmy.bitcast(F32)[:], 1.0)
                for _ in range(8):
                    pdmy = apq.tile([P, 4 * P], F32, tag="pq", name="pdmy")
                    nc.tensor.matmul(pdmy[:], dmy[:, :P], dmy[:, P:],
                                     start=True, stop=True)
                nc.scalar.dma_start(wq_sb[:], wq_d[:])
                nc.scalar.dma_start(wk_sb[:], wk_d[:])
                nc.scalar.dma_start(wv_sb[:], wv_d[:])
                # xkg streamed on gpsimd for all heads (chunked); am per head
                xg_tiles = {}
                for hh in range(HPC):
                    for ci, (t0, t1) in enumerate(plan):
                        w = (t1 - t0) * P
                        xg = pxg.tile([P, NKT, WMAX], BF16, tag="xkg")
                        nc.gpsimd.dma_start(
                            xg[:, :, :w], xkg_d[hh][:, :, t0 * P:t1 * P])
                        xg_tiles[(hh, ci)] = xg
                    am_t = amp.tile([P, NE, QCH], BF16, tag="am")
                    nc.gpsimd.dma_start(am_t[:], am_d[hh])
                    xg_tiles[(hh, "am")] = am_t

                for c in range(NXC):
                    if c == 4:
                        emit_a2(0)
                    xt = ax.tile([P, NKT, XCH], BF16, tag="xt")
                    nc.sync.dma_start(xt[:], xt_d[c])
                    for hh in range(HPC):
                        pq = apq.tile([P, XCH], F32, tag="pq", name="pq")
                        for kt in range(NKT):
                            nc.tensor.matmul(pq[:], wq_sb[:, hh, kt],
                                             xt[:, kt],
                                             start=(kt == 0),
                                             stop=(kt == NKT - 1))
                        nc.scalar.activation(
                            q_sb[:, hh, c * XCH:(c + 1) * XCH], pq[:],
                            AF.Identity, bias=bq_sb[:, hh:hh + 1])
            nc.scalar.dma_start(wo_sb[:], wo_d[:])

            # -------- Stage A2 (K/V compact) + Stage B (attention) ---
            with tc.tile_pool(name="bev", bufs=3) as bev, \
                 tc.tile_pool(name="bt", bufs=2) as bt, \
                 tc.tile_pool(name="bps", bufs=3, space="PSUM") as bps, \
                 tc.tile_pool(name="bpo", bufs=2, space="PSUM") as bpo, \
                 tc.tile_pool(name="bpr", bufs=1, space="PSUM") as bpr:

                def emit_epilogue(h, j, po, pr):
                    dst = ot_sb[:, h, j * QCH:(j + 1) * QCH]
                    if j in JD:
                        flag = bt.tile([P, QCH], F32, tag="flag")
                        nc.vector.tensor_scalar(flag[:], pr[:], 0.0, None,
                                                op0=OP.is_equal)
                        rs2 = bt.tile([P, QCH], F32, tag="rs2")
                        nc.vector.tensor_tensor(rs2[:], pr[:], flag[:],
                                                op=OP.add)
                        recip = bt.tile([P, QCH], F32, tag="recip")
                        nc.vector.reciprocal(recip[:], rs2[:])
                        o1 = bt.tile([P, QCH], F32, tag="o1")
                        nc.vector.tensor_tensor(o1[:], po[:], recip[:],
                                                op=OP.mult)
                        nc.vector.scalar_tensor_tensor(
                            dst, flag[:], vm_sb[:, h:h + 1], o1[:],
                            op0=OP.mult, op1=OP.add)
                    else:
                        recip = bt.tile([P, QCH], F32, tag="recip")
                        nc.vector.reciprocal(recip[:], pr[:])
                        nc.vector.tensor_tensor(dst, po[:], recip[:],
                                                op=OP.mult)

                pend = []

                def flush_one():
                    (po, pr, h, kt, ev, first, last, epi) = pend.pop(0)
                    nc.tensor.matmul(po[:], v_sb[:, h, kt, :], ev[:],
                                     start=first, stop=last)
                    nc.tensor.matmul(pr[:], ones_sb[:], ev[:],
                                     start=first, stop=last)
                    if epi is not None:
                        emit_epilogue(*epi)

                def emit_b(h):
                    am_t = xg_tiles[(h, "am")]
                    for j in range(NQCH):
                        tj = T[j]
                        if tj == 0:
                            continue
                        po = bpo.tile([P, QCH], F32, tag="po")
                        pr = bpr.tile([P, QCH], F32, tag="pr")
                        for kt in range(tj):
                            ps = bps.tile([P, QCH], F32, tag="ps")
                            nc.tensor.matmul(
                                ps[:], k_sb[:, h, kt * P:(kt + 1) * P],
                                q_sb[:, h, j * QCH:(j + 1) * QCH],
                                start=True, stop=True)
                            while len(pend) >= 2:
                                flush_one()
                            ev = bev.tile([P, QCH], BF16, tag="ev")
                            nc.scalar.activation(
                                ev[:], ps[:], AF.Exp,
                                bias=tpa_sb[:, h, kt:kt + 1],
                                scale=INV_SQRT_DH)
                            if (j, kt) in eidx:
                                evm = bev.tile([P, QCH], BF16, tag="evm")
                                nc.vector.tensor_tensor(
                                    evm[:], ev[:], am_t[:, eidx[(j, kt)], :],
                                    op=OP.mult)
                                ev = evm
                            epi = (h, j, po, pr) if kt == tj - 1 else None
                            pend.append((po, pr, h, kt, ev,
                                         kt == 0, kt == tj - 1, epi))

                emit_a2(1)
                emit_b(0)
                emit_a2(2)
                emit_b(1)
                emit_a2(3)
                emit_b(2)
                emit_b(3)
                while pend:
                    flush_one()
            apv_cm.__exit__(None, None, None)
            apk_cm.__exit__(None, None, None)

        # ---------------- Stage C: o_proj partial ----------------
        with tc.tile_pool(name="ccp", bufs=3) as ccp, \
             tc.tile_pool(name="cps", bufs=4, space="PSUM") as cps:
            for st in range(S // P):
                for ec in range(D // QCH):
                    pc = cps.tile([P, QCH], F32, tag="pc")
                    for h in range(HPC):
                        nc.tensor.matmul(
                            pc[:], ot_sb[:, h, st * P:(st + 1) * P],
                            wo_sb[:, h, ec * QCH:(ec + 1) * QCH],
                            start=(h == 0), stop=(h == HPC - 1))
                    osb = ccp.tile([P, QCH], BF16, tag="osb")
                    nc.scalar.activation(osb[:], pc[:], AF.Identity)
                    (nc.sync if ec % 2 == 0 else nc.gpsimd).dma_start(
                        out_d[st * P:(st + 1) * P,
                              ec * QCH:(ec + 1) * QCH], osb[:])

        if dbg:
            nc.sync.dma_start(qd_d[:], q_sb[:])
            nc.sync.dma_start(kd_d[:], k_sb[:])
            nc.sync.dma_start(vd_d[:], v_sb[:])
            nc.sync.dma_start(od_d[:], ot_sb[:])
        amp_cm.__exit__(None, None, None)
        pxg_cm.__exit__(None, None, None)
        pre_cm.__exit__(None, None, None)

    _fix_waits(nc, 1)
    return nc


def _host_mask_and_vmean(hidden_states, Wv, bv, Wdt, bdt, A, ratio_permille):
    """Dynamic-mask pipeline on host, bit-matched to the jax reference."""
    import jax
    import jax.numpy as jnp

    cpu = jax.devices("cpu")[0]
    with jax.default_device(cpu):
        hs = jnp.asarray(hidden_states, dtype=jnp.float32)
        v_lin = jnp.einsum('bsd,ed->bse', hs, jnp.asarray(Wv, jnp.float32)) \
            + jnp.asarray(bv, jnp.float32)
        dt = jnp.einsum('bsd,hd->bsh', v_lin, jnp.asarray(Wdt, jnp.float32)) \
            + jnp.asarray(bdt, jnp.float32)
        dyn = jnp.exp(jnp.asarray(A, jnp.float32) * jax.nn.softplus(dt))
        dynT = dyn.transpose(0, 2, 1)                       # [B, H, S]
        ratio = float(ratio_permille) / 1000.0
        num = int(S * ratio)
        if 0.0 < ratio < 1.0 and num > 0:
            kth = jnp.sort(dynT, axis=-1)[..., num - 1:num]
            tmask = jnp.where(dynT < kth, NEG, dynT)
        else:
            tmask = dynT
        vmean = v_lin.mean(axis=1)                          # [B, D]
        tmask = np.asarray(tmask, dtype=np.float32)
        vmean = np.asarray(vmean, dtype=np.float32)
    return np.maximum(tmask, np.float32(NEG)), vmean


def kernel(hidden_states, attention_mask, Wq, bq, Wk, bk, Wv, bv,
           Wdt, bdt, A, Wo, bo, ratio_permille):
    f32 = np.float32
    hidden_states = np.asarray(hidden_states, f32)
    attention_mask = np.asarray(attention_mask, f32)
    Wq, bq = np.asarray(Wq, f32), np.asarray(bq, f32)
    Wk, bk = np.asarray(Wk, f32), np.asarray(bk, f32)
    Wv, bv = np.asarray(Wv, f32), np.asarray(bv, f32)
    Wdt, bdt = np.asarray(Wdt, f32), np.asarray(bdt, f32)
    A_, Wo, bo = np.asarray(A, f32), np.asarray(Wo, f32), np.asarray(bo, f32)

    tmask, vmean = _host_mask_and_vmean(hidden_states, Wv, bv, Wdt, bdt, A_,
                                        ratio_permille)
    okb = attention_mask[:, 0] != np.float32(MIN32)         # [B, S, S] (q, k)

    # ---- shared program structure from the actual data ----
    survs = {}
    for b in range(B):
        for h in range(H):
            survs[(b, h)] = np.nonzero(tmask[b, h] > NEGT)[0]

    Tj = np.zeros(NQCH, np.int64)
    deg = np.zeros(NQCH, bool)
    for b in range(B):
        for h in range(H):
            sv = survs[(b, h)]
            okr = okb[b][:, sv] if sv.size else np.zeros((S, 0), bool)
            for j in range(NQCH):
                sub = okr[j * QCH:(j + 1) * QCH]
                anyv = sub.any(axis=0)
                nz = np.nonzero(anyv)[0]
                tc_ = 0 if nz.size == 0 else int(nz[-1]) // P + 1
                Tj[j] = max(Tj[j], tc_)
                if sub.shape[1] == 0 or not sub.any(axis=1).all():
                    deg[j] = True
    T = tuple(int(t) for t in Tj)
    NT = max(max(T), 1)
    KP = NT * P

    okg_pads = {}
    edge_need = [set() for _ in range(NQCH)]
    for b in range(B):
        for h in range(H):
            sv = survs[(b, h)]
            ns = sv.size
            svp = np.concatenate(
                [sv, np.full(KP - ns, sv[-1] if ns else 0, sv.dtype)])
            okg = np.ones((S, KP), bool)
            if ns:
                okg[:, :ns] = okb[b][:, sv]
            else:
                okg[:] = True
            okg_pads[(b, h)] = (svp, ns, okg)
            for j in range(NQCH):
                for kt in range(T[j]):
                    if not okg[j * QCH:(j + 1) * QCH, kt * P:(kt + 1) * P].all():
                        edge_need[j].add(kt)
    EDGE = tuple(tuple(sorted(e)) for e in edge_need)
    JD = tuple(int(j) for j in range(NQCH) if deg[j])
    edge_list = [(j, kt) for j in range(NQCH) for kt in EDGE[j]]
    NE = max(len(edge_list), 1)

    cfg = {"T": T, "NT": NT, "EDGE": EDGE, "JD": JD}
    key = (T, NT, EDGE, JD)
    if _prog_cache.get("key") != key:
        _prog_cache["nc"] = _build_program(cfg)
        _prog_cache["key"] = key
    nc = _prog_cache["nc"]

    ones_blk = np.ones((P, P), NPBF16)
    in_maps = []
    for c in range(NCORE):
        b, hg = divmod(c, HGRP)
        h0 = hg * HPC
        e0 = hg * HD
        x = hidden_states[b]                                 # [S, D]

        xt = np.ascontiguousarray(
            x.reshape(NXC, XCH, NKT, P).transpose(0, 3, 2, 1)).astype(NPBF16)
        xkg = np.empty((HPC, P, NKT, KP), NPBF16)
        tpa = np.empty((P, HPC, NT), f32)
        am_c = np.zeros((HPC, P, NE, QCH), NPBF16)
        for hh in range(HPC):
            h = h0 + hh
            svp, ns, okg = okg_pads[(b, h)]
            xg = x[svp]                                      # [KP, D]
            xkg[hh] = xg.reshape(KP, NKT, P).transpose(2, 1, 0).astype(NPBF16)
            vals = np.full(KP, NEG, f32)
            vals[:ns] = tmask[b, h, svp[:ns]]
            tpa[:, hh, :] = vals.reshape(NT, P).T
            for idx, (j, kt) in enumerate(edge_list):
                am_c[hh, :, idx, :] = \
                    okg[j * QCH:(j + 1) * QCH, kt * P:(kt + 1) * P].T

        def lhsfmt(W):
            t = W[e0:e0 + HD].reshape(HPC, P, NKT, P)
            return np.ascontiguousarray(t.transpose(3, 0, 2, 1)).astype(NPBF16)

        wq_c = lhsfmt(Wq)
        wk_c = lhsfmt(Wk)
        wv_c = lhsfmt(Wv)
        wo_c = np.ascontiguousarray(
            Wo[:, e0:e0 + HD].T.reshape(HPC, P, D)
            .transpose(1, 0, 2)).astype(NPBF16)
        bq_c = np.ascontiguousarray(bq[e0:e0 + HD].reshape(HPC, P).T)
        bk_c = np.ascontiguousarray(bk[e0:e0 + HD].reshape(HPC, P).T)
        bvb_c = np.ascontiguousarray(
            np.broadcast_to(bv[e0:e0 + HD], (P, HD))).astype(f32)
        vm_c = np.ascontiguousarray(vmean[b, e0:e0 + HD].reshape(HPC, P).T)

        in_maps.append({
            "xt": xt, "xkg": xkg, "wq": wq_c, "wk": wk_c, "wv": wv_c,
            "wo": wo_c, "am": am_c, "tp": tpa, "onesw": ones_blk,
            "bqv": bq_c, "bkv": bk_c, "bvb": bvb_c, "vm": vm_c,
        })

    res = run_bass_kernel_spmd(nc, in_maps, list(range(NCORE)))

    out = np.zeros((B, S, D), np.float64)
    for c in range(NCORE):
        b = c // HGRP
        out[b] += res.results[c]["opart"].astype(np.float64)
    out += bo.astype(np.float64)
    return out.astype(f32)


# revision 8
# speedup vs baseline: 1.3192x; 1.0103x over previous
"""Trainium2 Bass kernel for sparse-attention (nn_DMA_14903536517676).

Full (unsharded) inputs in, full output out. Shards across 8 NeuronCores:
data-parallel over batch (B=2) x tensor-parallel over heads (4 per core).

Key idea vs the dense baseline: the dynamic (kth-threshold) mask depends only
on the KEY position, killing ~half of all keys per (batch, head). The host
computes the surviving-key set per head and gathers the corresponding x
columns; the device then computes K/V projections and the whole attention
block only over compacted survivor slots (causal q-chunks touch only a
prefix of the survivor list). All matmul operands are bf16 (full PE rate at
any tile width in fp32-accumulate), Q/K/V stay SBUF-resident (no DRAM
scratch roundtrip), and the o_proj partial is written bf16 and reduced on
host. Dynamic-mask values ride the exp bias per survivor slot; the ragged
causal edge is a small set of host-built 0/1 tiles multiplied into ev.
"""

import math

import numpy as np
import ml_dtypes

import concourse.bass as bass
import concourse.mybir as mybir
import concourse.tile as tile
from concourse.bass_utils import run_bass_kernel_spmd

F32 = mybir.dt.float32
F32R = mybir.dt.float32r
BF16 = mybir.dt.bfloat16
NPBF16 = ml_dtypes.bfloat16
AF = mybir.ActivationFunctionType
OP = mybir.AluOpType

B, S, D, H = 2, 2048, 2048, 16
P = 128
DH = D // H            # 128
NCORE = 8
HGRP = NCORE // B      # 4 head-groups (cores) per batch element
HPC = H // HGRP        # 4 heads per core
HD = HPC * DH          # 512 head dims per core
NKT = D // P           # 16 contraction tiles over D
XCH = 256              # Q-projection s-chunk width
NXC = S // XCH         # 8
QCH = 512              # attention q-chunk width
NQCH = S // QCH        # 4
MIN32 = float(np.finfo(np.float32).min)
NEG = -1.0e37          # clamped mask sentinel (exp underflows to exact 0)
NEGT = -1.0e36         # threshold for "is masked" tests on host
INV_SQRT_DH = 1.0 / math.sqrt(DH)

_prog_cache = {}


# ---------------------------------------------------------------------------
# Walrus in this toolchain accepts at most ONE embedded sync-wait command per
# instruction. Tile emits more. Move overflow waits onto InstNoOp
# instructions inserted immediately before, on the same engine (semaphores
# are monotonic in this loop-free program, so waiting earlier is safe).
# ---------------------------------------------------------------------------
def _fix_waits(nc, maxw=1):
    uid = 0
    for f in nc.m.functions:
        for b in f.blocks:
            il = b.instructions
            idx = 0
            while idx < len(il):
                inst = il[idx]
                si = getattr(inst, "sync_info", None)
                if si is None:
                    idx += 1
                    continue
                waits = si.on_wait or []
                if len(waits) <= maxw:
                    idx += 1
                    continue
                si.on_wait = waits[-maxw:]
                overflow = waits[:-maxw]
                nops = []
                for j in range(0, len(overflow), maxw):
                    uid += 1
                    nop = mybir.InstNoOp(name=f"I-waitnop-{uid}")
                    nop.engine = inst.engine
                    nop.sync_info = mybir.SyncInfo(
                        on_wait=overflow[j:j + maxw], on_update=[])
                    nops.append(nop)
                for k, nop in enumerate(nops):
                    il.insert(idx + k, nop)
                idx += len(nops) + 1


def _chunk_plan(nt):
    """Split nt 128-wide tiles into DMA/compute chunks of 2 (last 3 if odd)."""
    if nt <= 3:
        return [(0, nt)]
    cuts = list(range(0, nt - 3, 2)) + [nt - 3 if nt % 2 else nt - 2]
    cuts = sorted(set(c for c in cuts if c >= 0))
    plan = []
    prev = 0
    for c in cuts[1:] + [nt]:
        plan.append((prev, c))
        prev = c
    return plan


def _build_program(cfg):
    T = cfg["T"]
    NT = cfg["NT"]
    KP = NT * P
    EDGE = cfg["EDGE"]
    JD = set(cfg["JD"])
    edge_list = [(j, kt) for j in range(NQCH) for kt in EDGE[j]]
    NE = max(len(edge_list), 1)
    eidx = {jk: i for i, jk in enumerate(edge_list)}

    nc = bass.Bass()

    xt_d = nc.declare_dram_parameter("xt", [NXC, P, NKT, XCH], BF16, isOutput=False)
    xkg_d = nc.declare_dram_parameter("xkg", [HPC, P, NKT, KP], BF16, isOutput=False)
    wq_d = nc.declare_dram_parameter("wq", [P, HPC, NKT, P], BF16, isOutput=False)
    wk_d = nc.declare_dram_parameter("wk", [P, HPC, NKT, P], BF16, isOutput=False)
    wv_d = nc.declare_dram_parameter("wv", [P, HPC, NKT, P], BF16, isOutput=False)
    wo_d = nc.declare_dram_parameter("wo", [P, HPC, D], BF16, isOutput=False)
    am_d = nc.declare_dram_parameter("am", [HPC, P, NE, QCH], BF16, isOutput=False)
    tpa_d = nc.declare_dram_parameter("tp", [P, HPC, NT], F32, isOutput=False)
    ones_d = nc.declare_dram_parameter("onesw", [P, P], BF16, isOutput=False)
    bq_d = nc.declare_dram_parameter("bqv", [P, HPC], F32, isOutput=False)
    bk_d = nc.declare_dram_parameter("bkv", [P, HPC], F32, isOutput=False)
    bvb_d = nc.declare_dram_parameter("bvb", [P, HD], F32, isOutput=False)
    vm_d = nc.declare_dram_parameter("vm", [P, HPC], F32, isOutput=False)
    out_d = nc.declare_dram_parameter("opart", [S, D], BF16, isOutput=True)
    dbg = cfg.get("dbg")
    if dbg:
        qd_d = nc.declare_dram_parameter("qdump", [P, HPC, S], BF16, isOutput=True)
        kd_d = nc.declare_dram_parameter("kdump", [P, HPC, KP], BF16, isOutput=True)
        vd_d = nc.declare_dram_parameter("vdump", [P, HPC, NT, P], BF16, isOutput=True)
        od_d = nc.declare_dram_parameter("otdump", [P, HPC, S], BF16, isOutput=True)

    plan = _chunk_plan(NT)
    WMAX = max(t1 - t0 for t0, t1 in plan) * P

    with tile.TileContext(nc) as tc:
        pre_cm = tc.tile_pool(name="pre", bufs=1)
        pre = pre_cm.__enter__()
        q_sb = pre.tile([P, HPC, S], BF16)
        k_sb = pre.tile([P, HPC, KP], BF16)
        v_sb = pre.tile([P, HPC, NT, P], BF16)
        ot_sb = pre.tile([P, HPC, S], BF16)
        wo_sb = pre.tile([P, HPC, D], BF16)
        ones_sb = pre.tile([P, P], BF16)
        tpa_sb = pre.tile([P, HPC, NT], F32)
        vm_sb = pre.tile([P, HPC], F32)
        bq_sb = pre.tile([P, HPC], F32)
        bk_sb = pre.tile([P, HPC], F32)
        bvb_sb = pre.tile([P, HD], F32)
        pxg_cm = tc.tile_pool(name="pxg", bufs=2)
        pxg = pxg_cm.__enter__()
        amp_cm = tc.tile_pool(name="amp", bufs=2)
        amp = amp_cm.__enter__()


        with tc.tile_pool(name="aw", bufs=1) as aw:
            wk_sb = aw.tile([P, HPC, NKT, P], BF16)
            wv_sb = aw.tile([P, HPC, NKT, P], BF16)

            apk_cm = tc.tile_pool(name="apk", bufs=1, space="PSUM")
            apk = apk_cm.__enter__()
            apv_cm = tc.tile_pool(name="apv", bufs=1, space="PSUM")
            apv = apv_cm.__enter__()

            def emit_a2(hh):
                for ci, (t0, t1) in enumerate(plan):
                    w = (t1 - t0) * P
                    xg = xg_tiles[(hh, ci)]
                    pk = apk.tile([P, WMAX], F32, tag="pk")
                    for kt in range(NKT):
                        nc.tensor.matmul(pk[:, :w], wk_sb[:, hh, kt],
                                         xg[:, kt, :w],
                                         start=(kt == 0),
                                         stop=(kt == NKT - 1))
                    nc.scalar.activation(
                        k_sb[:, hh, t0 * P:t1 * P], pk[:, :w],
                        AF.Identity, bias=bk_sb[:, hh:hh + 1])
                    for t in range(t0, t1):
                        pv = apv.tile([P, P], F32, tag="pv")
                        for kt in range(NKT):
                            nc.tensor.matmul(
                                pv[:],
                                xg[:, kt, (t - t0) * P:(t - t0 + 1) * P],
                                wv_sb[:, hh, kt],
                                start=(kt == 0), stop=(kt == NKT - 1))
                        nc.vector.scalar_tensor_tensor(
                            v_sb[:, hh, t, :], pv[:], 1.0,
                            bvb_sb[:, hh * P:(hh + 1) * P],
                            op0=OP.mult, op1=OP.add)

            # ------------- Stage A: Q projection (dense) -------------
            with tc.tile_pool(name="awq", bufs=1) as awq, \
                 tc.tile_pool(name="ax", bufs=2) as ax, \
                 tc.tile_pool(name="apq", bufs=3, space="PSUM") as apq:
                wq_sb = awq.tile([P, HPC, NKT, P], BF16)
                # dep-free PE warmup (ramps the p-state before real work)
                dmy = awq.tile([P, 5 * P], F32R)
                nc.vector.memset(dmy.bitcast(F32)[:], 1.0)
                for _ in range(8):
                    pdmy = apq.tile([P, 4 * P], F32, tag="pq", name="pdmy")
                    nc.tensor.matmul(pdmy[:], dmy[:, :P], dmy[:, P:],
                                     start=True, stop=True)
                nc.scalar.dma_start(wq_sb[:], wq_d[:])
                nc.scalar.dma_start(bq_sb[:], bq_d[:])
                nc.scalar.dma_start(wk_sb[:], wk_d[:])
                nc.scalar.dma_start(wv_sb[:], wv_d[:])
                nc.scalar.dma_start(tpa_sb[:], tpa_d[:])
                nc.scalar.dma_start(vm_sb[:], vm_d[:])
                nc.scalar.dma_start(bk_sb[:], bk_d[:])
                nc.scalar.dma_start(bvb_sb[:], bvb_d[:])
                nc.sync.dma_start(ones_sb[:], ones_d[:])
                # xkg streamed on gpsimd for all heads (chunked); am per head
                xg_tiles = {}
                for hh in range(HPC):
                    for ci, (t0, t1) in enumerate(plan):
                        w = (t1 - t0) * P
                        xg = pxg.tile([P, NKT, WMAX], BF16, tag="xkg")
                        nc.gpsimd.dma_start(
                            xg[:, :, :w], xkg_d[hh][:, :, t0 * P:t1 * P])
                        xg_tiles[(hh, ci)] = xg
                    am_t = amp.tile([P, NE, QCH], BF16, tag="am")
                    nc.gpsimd.dma_start(am_t[:], am_d[hh])
                    xg_tiles[(hh, "am")] = am_t

                for c in range(NXC):
                    if c == 4:
                        emit_a2(0)
                    if c == 7:
                        emit_a2(1)
                    xt = ax.tile([P, NKT, XCH], BF16, tag="xt")
                    nc.sync.dma_start(xt[:], xt_d[c])
                    for hh in range(HPC):
                        pq = apq.tile([P, XCH], F32, tag="pq", name="pq")
                        for kt in range(NKT):
                            nc.tensor.matmul(pq[:], wq_sb[:, hh, kt],
                                             xt[:, kt],
                                             start=(kt == 0),
                                             stop=(kt == NKT - 1))
                        nc.scalar.activation(
                            q_sb[:, hh, c * XCH:(c + 1) * XCH], pq[:],
                            AF.Identity, bias=bq_sb[:, hh:hh + 1])
            nc.scalar.dma_start(wo_sb[:], wo_d[:])

            # -------- Stage A2 (K/V compact) + Stage B (attention) ---
            with tc.tile_pool(name="bev", bufs=3) as bev, \
                 tc.tile_pool(name="bt", bufs=2) as bt, \
                 tc.tile_pool(name="bps", bufs=3, space="PSUM") as bps, \
                 tc.tile_pool(name="bpo", bufs=2, space="PSUM") as bpo, \
                 tc.tile_pool(name="bpr", bufs=1, space="PSUM") as bpr:

                def emit_epilogue(h, j, po, pr):
                    dst = ot_sb[:, h, j * QCH:(j + 1) * QCH]
                    if j in JD:
                        flag = bt.tile([P, QCH], F32, tag="flag")
                        nc.vector.tensor_scalar(flag[:], pr[:], 0.0, None,
                                                op0=OP.is_equal)
                        rs2 = bt.tile([P, QCH], F32, tag="rs2")
                        nc.vector.tensor_tensor(rs2[:], pr[:], flag[:],
                                                op=OP.add)
                        recip = bt.tile([P, QCH], F32, tag="recip")
                        nc.vector.reciprocal(recip[:], rs2[:])
                        o1 = bt.tile([P, QCH], F32, tag="o1")
                        nc.vector.tensor_tensor(o1[:], po[:], recip[:],
                                                op=OP.mult)
                        nc.vector.scalar_tensor_tensor(
                            dst, flag[:], vm_sb[:, h:h + 1], o1[:],
                            op0=OP.mult, op1=OP.add)
                    else:
                        recip = bt.tile([P, QCH], F32, tag="recip")
                        nc.vector.reciprocal(recip[:], pr[:])
                        nc.vector.tensor_tensor(dst, po[:], recip[:],
                                                op=OP.mult)

                pend = []

                def flush_one():
                    (po, pr, h, kt, ev, first, last, epi) = pend.pop(0)
                    nc.tensor.matmul(pr[:], ones_sb[:], ev[:],
                                     start=first, stop=last)
                    nc.tensor.matmul(po[:], v_sb[:, h, kt, :], ev[:],
                                     start=first, stop=last)
                    if epi is not None:
                        emit_epilogue(*epi)

                def emit_b(h):
                    am_t = xg_tiles[(h, "am")]
                    for j in range(NQCH):
                        tj = T[j]
                        if tj == 0:
                            continue
                        po = bpo.tile([P, QCH], F32, tag="po")
                        pr = bpr.tile([P, QCH], F32, tag="pr")
                        for kt in range(tj):
                            ps = bps.tile([P, QCH], F32, tag="ps")
                            nc.tensor.matmul(
                                ps[:], k_sb[:, h, kt * P:(kt + 1) * P],
                                q_sb[:, h, j * QCH:(j + 1) * QCH],
                                start=True, stop=True)
                            while len(pend) >= 2:
                                flush_one()
                            ev = bev.tile([P, QCH], BF16, tag="ev")
                            nc.scalar.activation(
                                ev[:], ps[:], AF.Exp,
                                bias=tpa_sb[:, h, kt:kt + 1],
                                scale=INV_SQRT_DH)
                            if (j, kt) in eidx:
                                evm = bev.tile([P, QCH], BF16, tag="evm")
                                nc.vector.tensor_tensor(
                                    evm[:], ev[:], am_t[:, eidx[(j, kt)], :],
                                    op=OP.mult)
                                ev = evm
                            epi = (h, j, po, pr) if kt == tj - 1 else None
                            pend.append((po, pr, h, kt, ev,
                                         kt == 0, kt == tj - 1, epi))

                emit_b(0)
                emit_a2(2)
                emit_b(1)
                emit_a2(3)
                emit_b(2)
                emit_b(3)
                while pend:
                    flush_one()
            apv_cm.__exit__(None, None, None)
            apk_cm.__exit__(None, None, None)

        # ---------------- Stage C: o_proj partial ----------------
        with tc.tile_pool(name="ccp", bufs=3) as ccp, \
             tc.tile_pool(name="cps", bufs=4, space="PSUM") as cps:
            for st in range(S // P):
                for ec in range(D // QCH):
                    pc = cps.tile([P, QCH], F32, tag="pc")
                    for h in range(HPC):
                        nc.tensor.matmul(
                            pc[:], ot_sb[:, h, st * P:(st + 1) * P],
                            wo_sb[:, h, ec * QCH:(ec + 1) * QCH],
                            start=(h == 0), stop=(h == HPC - 1))
                    osb = ccp.tile([P, QCH], BF16, tag="osb")
                    nc.scalar.activation(osb[:], pc[:], AF.Identity)
                    (nc.sync if ec % 2 == 0 else nc.gpsimd).dma_start(
                        out_d[st * P:(st + 1) * P,
                              ec * QCH:(ec + 1) * QCH], osb[:])

        if dbg:
            nc.sync.dma_start(qd_d[:], q_sb[:])
            nc.sync.dma_start(kd_d[:], k_sb[:])
            nc.sync.dma_start(vd_d[:], v_sb[:])
            nc.sync.dma_start(od_d[:], ot_sb[:])
        amp_cm.__exit__(None, None, None)
        pxg_cm.__exit__(None, None, None)
        pre_cm.__exit__(None, None, None)

    _fix_waits(nc, 1)
    return nc


def _host_mask_and_vmean(hidden_states, Wv, bv, Wdt, bdt, A, ratio_permille):
    """Dynamic-mask pipeline on host, bit-matched to the jax reference."""
    import jax
    import jax.numpy as jnp

    cpu = jax.devices("cpu")[0]
    with jax.default_device(cpu):
        hs = jnp.asarray(hidden_states, dtype=jnp.float32)
        v_lin = jnp.einsum('bsd,ed->bse', hs, jnp.asarray(Wv, jnp.float32)) \
            + jnp.asarray(bv, jnp.float32)
        dt = jnp.einsum('bsd,hd->bsh', v_lin, jnp.asarray(Wdt, jnp.float32)) \
            + jnp.asarray(bdt, jnp.float32)
        dyn = jnp.exp(jnp.asarray(A, jnp.float32) * jax.nn.softplus(dt))
        dynT = dyn.transpose(0, 2, 1)                       # [B, H, S]
        ratio = float(ratio_permille) / 1000.0
        num = int(S * ratio)
        if 0.0 < ratio < 1.0 and num > 0:
            kth = jnp.sort(dynT, axis=-1)[..., num - 1:num]
            tmask = jnp.where(dynT < kth, NEG, dynT)
        else:
            tmask = dynT
        vmean = v_lin.mean(axis=1)                          # [B, D]
        tmask = np.asarray(tmask, dtype=np.float32)
        vmean = np.asarray(vmean, dtype=np.float32)
    return np.maximum(tmask, np.float32(NEG)), vmean


def kernel(hidden_states, attention_mask, Wq, bq, Wk, bk, Wv, bv,
           Wdt, bdt, A, Wo, bo, ratio_permille):
    f32 = np.float32
    hidden_states = np.asarray(hidden_states, f32)
    attention_mask = np.asarray(attention_mask, f32)
    Wq, bq = np.asarray(Wq, f32), np.asarray(bq, f32)
    Wk, bk = np.asarray(Wk, f32), np.asarray(bk, f32)
    Wv, bv = np.asarray(Wv, f32), np.asarray(bv, f32)
    Wdt, bdt = np.asarray(Wdt, f32), np.asarray(bdt, f32)
    A_, Wo, bo = np.asarray(A, f32), np.asarray(Wo, f32), np.asarray(bo, f32)

    tmask, vmean = _host_mask_and_vmean(hidden_states, Wv, bv, Wdt, bdt, A_,
                                        ratio_permille)
    okb = attention_mask[:, 0] != np.float32(MIN32)         # [B, S, S] (q, k)

    # ---- shared program structure from the actual data ----
    survs = {}
    for b in range(B):
        for h in range(H):
            survs[(b, h)] = np.nonzero(tmask[b, h] > NEGT)[0]

    Tj = np.zeros(NQCH, np.int64)
    deg = np.zeros(NQCH, bool)
    for b in range(B):
        for h in range(H):
            sv = survs[(b, h)]
            okr = okb[b][:, sv] if sv.size else np.zeros((S, 0), bool)
            for j in range(NQCH):
                sub = okr[j * QCH:(j + 1) * QCH]
                anyv = sub.any(axis=0)
                nz = np.nonzero(anyv)[0]
                tc_ = 0 if nz.size == 0 else int(nz[-1]) // P + 1
                Tj[j] = max(Tj[j], tc_)
                if sub.shape[1] == 0 or not sub.any(axis=1).all():
                    deg[j] = True
    T = tuple(int(t) for t in Tj)
    NT = max(max(T), 1)
    KP = NT * P

    okg_pads = {}
    edge_need = [set() for _ in range(NQCH)]
    for b in range(B):
        for h in range(H):
            sv = survs[(b, h)]
            ns = sv.size
            svp = np.concatenate(
                [sv, np.full(KP - ns, sv[-1] if ns else 0, sv.dtype)])
            okg = np.ones((S, KP), bool)
            if ns:
                okg[:, :ns] = okb[b][:, sv]
            else:
                okg[:] = True
            okg_pads[(b, h)] = (svp, ns, okg)
            for j in range(NQCH):
                for kt in range(T[j]):
                    if not okg[j * QCH:(j + 1) * QCH, kt * P:(kt + 1) * P].all():
                        edge_need[j].add(kt)
    EDGE = tuple(tuple(sorted(e)) for e in edge_need)
    JD = tuple(int(j) for j in range(NQCH) if deg[j])
    edge_list = [(j, kt) for j in range(NQCH) for kt in EDGE[j]]
    NE = max(len(edge_list), 1)

    cfg = {"T": T, "NT": NT, "EDGE": EDGE, "JD": JD}
    key = (T, NT, EDGE, JD)
    if _prog_cache.get("key") != key:
        _prog_cache["nc"] = _build_program(cfg)
        _prog_cache["key"] = key
    nc = _prog_cache["nc"]

    ones_blk = np.ones((P, P), NPBF16)
    in_maps = []
    for c in range(NCORE):
        b, hg = divmod(c, HGRP)
        h0 = hg * HPC
        e0 = hg * HD
        x = hidden_states[b]                                 # [S, D]

        xt = np.ascontiguousarray(
            x.reshape(NXC, XCH, NKT, P).transpose(0, 3, 2, 1)).astype(NPBF16)
        xkg = np.empty((HPC, P, NKT, KP), NPBF16)
        tpa = np.empty((P, HPC, NT), f32)
        am_c = np.zeros((HPC, P, NE, QCH), NPBF16)
        for hh in range(HPC):
            h = h0 + hh
            svp, ns, okg = okg_pads[(b, h)]
            xg = x[svp]                                      # [KP, D]
            xkg[hh] = xg.reshape(KP, NKT, P).transpose(2, 1, 0).astype(NPBF16)
            vals = np.full(KP, NEG, f32)
            vals[:ns] = tmask[b, h, svp[:ns]]
            tpa[:, hh, :] = vals.reshape(NT, P).T
            for idx, (j, kt) in enumerate(edge_list):
                am_c[hh, :, idx, :] = \
                    okg[j * QCH:(j + 1) * QCH, kt * P:(kt + 1) * P].T

        def lhsfmt(W):
            t = W[e0:e0 + HD].reshape(HPC, P, NKT, P)
            return np.ascontiguousarray(t.transpose(3, 0, 2, 1)).astype(NPBF16)

        wq_c = lhsfmt(Wq)
        wk_c = lhsfmt(Wk)
        wv_c = lhsfmt(Wv)
        wo_c = np.ascontiguousarray(
            Wo[:, e0:e0 + HD].T.reshape(HPC, P, D)
            .transpose(1, 0, 2)).astype(NPBF16)
        bq_c = np.ascontiguousarray(bq[e0:e0 + HD].reshape(HPC, P).T)
        bk_c = np.ascontiguousarray(bk[e0:e0 + HD].reshape(HPC, P).T)
        bvb_c = np.ascontiguousarray(
            np.broadcast_to(bv[e0:e0 + HD], (P, HD))).astype(f32)
        vm_c = np.ascontiguousarray(vmean[b, e0:e0 + HD].reshape(HPC, P).T)

        in_maps.append({
            "xt": xt, "xkg": xkg, "wq": wq_c, "wk": wk_c, "wv": wv_c,
            "wo": wo_c, "am": am_c, "tp": tpa, "onesw": ones_blk,
            "bqv": bq_c, "bkv": bk_c, "bvb": bvb_c, "vm": vm_c,
        })

    res = run_bass_kernel_spmd(nc, in_maps, list(range(NCORE)))

    out = np.zeros((B, S, D), np.float64)
    for c in range(NCORE):
        b = c // HGRP
        out[b] += res.results[c]["opart"].astype(np.float64)
    out += bo.astype(np.float64)
    return out.astype(f32)
